# revision 18
# baseline (speedup 1.0000x reference)
"""KGCN (2-hop, 16-neighbor, relation-attention GNN) forward on 8 Trainium2 NeuronCores.

Data-parallel over batch (512 rows/core); tables replicated per core.

The dominant cost is ~140k random embedding-row fetches per core. SWDGE
descriptor generation on the GPSIMD engine costs ~7ns/descriptor no matter
the instruction, so the kernel minimizes per-instruction overhead by using
InstDMAGatherAnt (dma_gather) with 1024 descriptors per instruction spread
over 4 SWDGE queues, instead of one-index-per-partition indirect DMAs
(~1.15us per 128 descriptors).

dma_gather takes int16 indices, so tables > 32768 rows are host-packed 4
rows per table row (index = id>>2 < 27500; 512B descriptors):
  - e4:   entity embeddings [27500, 4*64] bf16 (the wanted 64-f32 sub-row is
          selected by folding a (s == id&3) one-hot into aggregation weights)
  - acmb: adj_ent||adj_rel combo [27500, 4*16 + 4*16] int32 (one gather
          fetches both neighbor ids and relation ids; sub-row selected on DVE)

dma_gather's index layout is (s p)-wrapped over 16 partitions and replicated
x8 (one copy per Q7 core): gather i reads idx[i%16, i//16] and writes output
partition i%128. Tokens are ordered (slot, batch-row) so output partition ==
batch row; the wrapped index buffers are built with PE-transpose pipelines
(partition-crossing element shuffles are only free on the PE).

Execution is software-pipelined per 128-row b-tile: the front end of b-tile
i+1 (adjacency fetch, index wrapping, score selection) is emitted before the
heavy phase of b-tile i (32 embedding gathers + weighted aggregation +
linear), so the GPSIMD engine streams descriptors continuously.

u/v/user lookups (tiny, f32-exact) stay on the old indirect-DMA path.
All index arithmetic runs in f32 (exact for ints < 2^24): sub-row s = x mod
4, packed row = (x - s)/4.
"""

import sys

sys.path.insert(0, "/opt/trn_rl_repo")

from contextlib import ExitStack

import numpy as np

import concourse.bass as bass
import concourse.mybir as mybir
import concourse.tile as tile
from concourse import bacc
from concourse.bass_utils import run_bass_kernel_spmd
from concourse.masks import make_identity

F32 = mybir.dt.float32
BF16 = mybir.dt.bfloat16
I32 = mybir.dt.int32
I16 = mybir.dt.int16
AF = mybir.ActivationFunctionType
ALU = mybir.AluOpType

N_CORES = 8
BATCH = 4096
BL = BATCH // N_CORES  # 512 batch rows per core
P = 128  # partitions
NT = BL // P  # 4 b-tiles per core
K = 16  # neighbors per node
D = 64  # embedding dim
R = 32  # num relations
TOTAL = 110000  # entity table rows (users + entities)
PACK = 4
PROWS = TOTAL // PACK  # 27500 packed rows
NQ = 4  # SWDGE queues
GIDX = 1024  # descriptors per dma_gather instruction (HW ring limit)


def build_program(total=TOTAL, bl=BL, spread_queues=True):
    nt = bl // P
    prows = total // PACK
    nc = bacc.Bacc("TRN2", target_bir_lowering=False, num_swdge_queues=NQ)

    u_d = nc.dram_tensor("u32", [bl], I32, kind="ExternalInput")
    v_d = nc.dram_tensor("v32", [bl], I32, kind="ExternalInput")
    vq_d = nc.dram_tensor("vq32", [bl], I32, kind="ExternalInput")
    sv_d = nc.dram_tensor("svf", [bl], F32, kind="ExternalInput")
    acmb_d = nc.dram_tensor("acmb", [prows, 8 * K], I32, kind="ExternalInput")
    e4_d = nc.dram_tensor("e4", [prows, PACK * D], BF16, kind="ExternalInput")
    ent_d = nc.dram_tensor("ent", [total, D], F32, kind="ExternalInput")
    relT_d = nc.dram_tensor("relT", [D, R], F32, kind="ExternalInput")
    wt_d = nc.dram_tensor("Wt", [D, D], F32, kind="ExternalInput")
    bias_d = nc.dram_tensor("bias", [D], F32, kind="ExternalInput")
    out_d = nc.dram_tensor("out", [bl], F32, kind="ExternalOutput")

    def old_gather(out_ap, table_ap, idx_ap):
        nc.gpsimd.indirect_dma_start(
            out=out_ap,
            out_offset=None,
            in_=table_ap,
            in_offset=bass.IndirectOffsetOnAxis(ap=idx_ap, axis=0),
        )

    def fat_gather(out_ap, table_ap, idx_ap, n_idx, elem):
        nc.gpsimd.dma_gather(
            out_ap=out_ap,
            in_ap=table_ap,
            idxs_ap=idx_ap,
            num_idxs=n_idx,
            num_idxs_reg=n_idx,
            elem_size=elem,
            queue_num=0,
        )

    with ExitStack() as ctx:
        tc = ctx.enter_context(tile.TileContext(nc))
        const = ctx.enter_context(tc.tile_pool(name="const", bufs=1))
        persist = ctx.enter_context(tc.tile_pool(name="persist", bufs=1))
        idxp = ctx.enter_context(tc.tile_pool(name="idxp", bufs=2))
        wrk = ctx.enter_context(tc.tile_pool(name="wrk", bufs=2))
        strm2 = ctx.enter_context(tc.tile_pool(name="strm2", bufs=2))
        strm1 = ctx.enter_context(tc.tile_pool(name="strm1", bufs=1))
        gat = ctx.enter_context(tc.tile_pool(name="gat", bufs=3))
        wev = ctx.enter_context(tc.tile_pool(name="wev", bufs=2))
        psA = ctx.enter_context(tc.tile_pool(name="psA", bufs=3, space="PSUM"))
        psM = ctx.enter_context(tc.tile_pool(name="psM", bufs=2, space="PSUM"))

        # ---- constants ----
        ident = const.tile([P, P], F32)
        make_identity(nc, ident[:])
        ones64 = const.tile([D, 1], F32)
        nc.vector.memset(ones64[:], 1.0)
        wt_sb = const.tile([D, D], F32)
        nc.sync.dma_start(out=wt_sb[:], in_=wt_d[:])
        relT_sb = const.tile([D, R], F32)
        nc.sync.dma_start(out=relT_sb[:], in_=relT_d[:])
        bias_sb = const.tile([D, 1], F32)
        nc.sync.dma_start(out=bias_sb[:], in_=bias_d.rearrange("(d one) -> d one", one=1))

        # ---- persistent (small) ----
        escb = [persist.tile([P, R], F32, name=f"escb_{i}") for i in range(nt)]
        userT = persist.tile([D, bl], F32, tag="userT")
        xfT = persist.tile([D, bl], F32, tag="xfT")

        def wrap16(src_f32_cols, dst_i16, ncols):
            """dst[q, t*8+g] = src[16g+q, t] (the (s p)-wrapped idx layout)."""
            nchunk = (ncols + 127) // 128
            for c in range(nchunk):
                w = min(128, ncols - c * 128)
                pt = psA.tile([128, P], F32, tag="t")
                nc.tensor.transpose(
                    pt[:w, :], src_f32_cols[:, c * 128 : c * 128 + w], ident[:]
                )
                tcs = wrk.tile([128, P], F32, tag="wrTc")
                nc.scalar.activation(tcs[:w, :], pt[:w, :], AF.Copy)
                for g in range(8):
                    pg = psA.tile([128, P], F32, tag="t")
                    nc.tensor.transpose(
                        pg[:16, :w], tcs[:w, g * 16 : (g + 1) * 16], ident[:w, :w]
                    )
                    st = c * 1024 + g
                    nc.vector.tensor_copy(
                        dst_i16[0:16, st : st + (w - 1) * 8 + 1 : 8],
                        pg[:16, :w],
                    )

        def replicate8(dst_128, src_16, width):
            v = dst_128.rearrange("(r q) c -> r q c", q=16)
            for g in range(8):
                nc.sync.dma_start(out=v[g], in_=src_16[0:16, 0:width])

        # ================= pass A: user embeddings -> userT =================
        for i in range(nt):
            sl = slice(i * P, (i + 1) * P)
            uidx = idxp.tile([P, 1], I32, tag="uidx")
            nc.sync.dma_start(
                out=uidx[:], in_=u_d[sl].rearrange("(p one) -> p one", one=1)
            )
            user_g = wrk.tile([P, D], F32, tag="user_g")
            old_gather(user_g[:], ent_d[:], uidx[:, 0:1])
            pst = psA.tile([128, P], F32, tag="t")
            nc.tensor.transpose(pst[:D, :], user_g[:], ident[:])
            nc.scalar.activation(userT[:, sl], pst[:D, :], AF.Copy)

        # ================= phase 2: relation scores =================
        ps = psM.tile([R, bl], F32, tag="mm")
        nc.tensor.matmul(ps[:], lhsT=relT_sb[:], rhs=userT[:], start=True, stop=True)
        esc_sb = wrk.tile([R, bl], F32, tag="esc_sb")
        nc.scalar.activation(esc_sb[:], ps[:], AF.Exp)
        for i in range(nt):
            pe = psA.tile([128, P], F32, tag="t")
            nc.tensor.transpose(pe[:, :R], esc_sb[:, i * P : (i + 1) * P], ident[:R, :R])
            nc.scalar.activation(escb[i][:], pe[:, :R], AF.Copy)

        # state handed from frontend(i) to heavy(i)
        st = [dict() for _ in range(nt)]

        def frontend(i):
            sl = slice(i * P, (i + 1) * P)
            S = st[i]

            vidx = idxp.tile([P, 1], I32, tag="vidx")
            nc.sync.dma_start(
                out=vidx[:], in_=v_d[sl].rearrange("(p one) -> p one", one=1)
            )
            ev0 = strm2.tile([P, D], F32, tag="ev0")
            old_gather(ev0[:], ent_d[:], vidx[:, 0:1])
            S["ev0"] = ev0

            vq = idxp.tile([P, 1], I32, tag="vq")
            nc.sync.dma_start(
                out=vq[:], in_=vq_d[sl].rearrange("(p one) -> p one", one=1)
            )
            vadj = wrk.tile([P, 8 * K], I32, tag="vadj")
            old_gather(vadj[:], acmb_d[:], vq[:, 0:1])
            svt = idxp.tile([P, 1], F32, tag="svt")
            nc.sync.dma_start(
                out=svt[:], in_=sv_d[sl].rearrange("(p one) -> p one", one=1)
            )

            r01 = strm2.tile([P, K + K * K], F32, tag="r01")
            r0f = r01[:, 0:K]
            r1f = r01[:, K : K + K * K]
            vadjf = wrk.tile([P, 8 * K], F32, tag="vadjf")
            nc.vector.tensor_copy(vadjf[:], vadj[:])
            e1x = wrk.tile([P, K], F32, tag="e1x")
            nc.vector.memset(e1x[:], 0.0)
            nc.vector.memset(r0f, 0.0)
            for s in range(PACK):
                m = wrk.tile([P, 1], F32, tag="svm")
                nc.vector.tensor_scalar(
                    out=m[:], in0=svt[:], scalar1=float(s), scalar2=None,
                    op0=ALU.is_equal,
                )
                nc.vector.scalar_tensor_tensor(
                    out=e1x[:], in0=vadjf[:, s * K : (s + 1) * K], scalar=m[:, 0:1],
                    in1=e1x[:], op0=ALU.mult, op1=ALU.add,
                )
                nc.vector.scalar_tensor_tensor(
                    out=r0f, in0=vadjf[:, 4 * K + s * K : 4 * K + (s + 1) * K],
                    scalar=m[:, 0:1], in1=r0f, op0=ALU.mult, op1=ALU.add,
                )

            e1i = wrk.tile([P, K], I32, tag="e1i")
            nc.vector.tensor_copy(e1i[:], e1x[:])
            s1i = wrk.tile([P, K], I32, tag="s1i")
            nc.vector.tensor_scalar(
                out=s1i[:], in0=e1i[:], scalar1=3, scalar2=None, op0=ALU.bitwise_and
            )
            s1f = wrk.tile([P, K], F32, tag="s1f")
            nc.vector.tensor_copy(s1f[:], s1i[:])
            e1qi = wrk.tile([P, K], I32, tag="e1qi")
            nc.vector.tensor_scalar(
                out=e1qi[:], in0=e1i[:], scalar1=2, scalar2=None,
                op0=ALU.logical_shift_right,
            )
            e1q = wrk.tile([P, K], F32, tag="e1q")
            nc.vector.tensor_copy(e1q[:], e1qi[:])

            wr1 = wrk.tile([16, K * 8], I16, tag="wr1")
            wrap16(e1q[:], wr1[:], K)
            rep1 = strm2.tile([P, K * 8], I16, tag="rep1")
            replicate8(rep1[:], wr1[:], K * 8)

            eadj = strm2.tile([P, K, 8 * K], I32, tag="eadj")
            ev1p = strm2.tile([P, K, PACK * D], BF16, tag="ev1p")
            for h in range(2):
                fat_gather(
                    eadj[:, h * 8 : (h + 1) * 8, :], acmb_d[:],
                    rep1[:, h * 64 : (h + 1) * 64], GIDX, 8 * K,
                )
            for h in range(2):
                fat_gather(
                    ev1p[:, h * 8 : (h + 1) * 8, :], e4_d[:],
                    rep1[:, h * 64 : (h + 1) * 64], GIDX, PACK * D,
                )

            # 4-pack select in int32, straight off the gathered rows
            e2acc = wrk.tile([P, K * K], I32, tag="e2acc")
            r1acc = wrk.tile([P, K * K], I32, tag="r1acc")
            nc.gpsimd.memset(e2acc[:], 0)
            nc.gpsimd.memset(r1acc[:], 0)
            e2v = e2acc[:].rearrange("p (m n) -> p m n", n=K)
            r1v = r1acc[:].rearrange("p (m n) -> p m n", n=K)
            for s in range(PACK):
                mi = wrk.tile([P, K], I32, tag="s1mi")
                nc.vector.tensor_scalar(
                    out=mi[:], in0=s1i[:], scalar1=s, scalar2=None,
                    op0=ALU.is_equal,
                )
                t = wrk.tile([P, K, K], I32, tag="selt")
                nc.gpsimd.tensor_tensor(
                    out=t[:], in0=eadj[:, :, s * K : (s + 1) * K],
                    in1=mi[:].broadcast_to([P, K, K]), op=ALU.mult,
                )
                nc.gpsimd.tensor_tensor(out=e2v, in0=e2v, in1=t[:], op=ALU.add)
                nc.gpsimd.tensor_tensor(
                    out=t[:], in0=eadj[:, :, 4 * K + s * K : 4 * K + (s + 1) * K],
                    in1=mi[:].broadcast_to([P, K, K]), op=ALU.mult,
                )
                nc.gpsimd.tensor_tensor(out=r1v, in0=r1v, in1=t[:], op=ALU.add)
            nc.gpsimd.tensor_copy(r1f, r1acc[:])

            e2i = e2acc
            s2i = wrk.tile([P, K * K], I32, tag="s2i")
            nc.vector.tensor_scalar(
                out=s2i[:], in0=e2i[:], scalar1=3, scalar2=None, op0=ALU.bitwise_and
            )
            s2 = wrk.tile([P, K * K], F32, tag="s2")
            nc.vector.tensor_copy(s2[:], s2i[:])
            e2qi = wrk.tile([P, K * K], I32, tag="e2qi")
            nc.vector.tensor_scalar(
                out=e2qi[:], in0=e2i[:], scalar1=2, scalar2=None,
                op0=ALU.logical_shift_right,
            )
            e2q = wrk.tile([P, K * K], F32, tag="e2q")
            nc.vector.tensor_copy(e2q[:], e2qi[:])
            wr2 = wrk.tile([16, 2048], I16, tag="wr2")
            wrap16(e2q[:], wr2[:], K * K)
            rep2 = strm2.tile([P, 2048], I16, tag="rep2")
            replicate8(rep2[:], wr2[:], 2048)
            S["rep2"] = rep2

            # aggregation weights: one-hot(s2) masks (esc factor applied below)
            w4t = strm2.tile([P, K * K, PACK], BF16, tag="w4t")
            for s in range(PACK):
                m = wrk.tile([P, K * K], F32, tag="s2m")
                nc.vector.tensor_scalar(
                    out=m[:], in0=s2[:], scalar1=float(s), scalar2=None,
                    op0=ALU.is_equal,
                )
                nc.scalar.activation(w4t[:, :, s], m[:], AF.Copy)
            S["w4t"] = w4t

            # ev1 selected embeddings
            w1 = wrk.tile([P, K, PACK], BF16, tag="w1")
            for s in range(PACK):
                m = wrk.tile([P, K], F32, tag="s1m2")
                nc.vector.tensor_scalar(
                    out=m[:], in0=s1f[:], scalar1=float(s), scalar2=None,
                    op0=ALU.is_equal,
                )
                nc.scalar.activation(w1[:, :, s], m[:], AF.Copy)
            wv1 = strm1.tile([P, K * PACK, D], BF16, tag="wv1")
            nc.vector.tensor_tensor(
                out=wv1[:],
                in0=ev1p[:].rearrange("p m (f d) -> p (m f) d", d=D),
                in1=w1[:].rearrange("p m f -> p (m f)").broadcast_to([P, K * PACK, D]),
                op=ALU.mult,
            )
            ev1s = strm2.tile([P, K, D], F32, tag="ev1s")
            nc.vector.tensor_reduce(
                out=ev1s[:],
                in_=wv1[:].rearrange("p (m f) d -> p m d f", f=PACK),
                axis=mybir.AxisListType.X,
                op=ALU.add,
            )
            S["ev1s"] = ev1s

            # esc selection + denominators (esc0 and esc1 in one 272-wide pass)
            esc01 = strm2.tile([P, K + K * K], F32, tag="esc01")
            esc0 = esc01[:, 0:K]
            esc1 = esc01[:, K : K + K * K]
            nc.vector.memset(esc01[:], 0.0)
            for r in range(R):
                m01 = wrk.tile([P, K + K * K], F32, tag="m01")
                nc.vector.tensor_scalar(
                    out=m01[:], in0=r01[:], scalar1=float(r), scalar2=None,
                    op0=ALU.is_equal,
                )
                nc.vector.scalar_tensor_tensor(
                    out=esc01[:], in0=m01[:], scalar=escb[i][:, r : r + 1],
                    in1=esc01[:], op0=ALU.mult, op1=ALU.add,
                )
            S["esc0"] = esc0
            den0 = wrk.tile([P, 1], F32, tag="den0")
            nc.vector.tensor_reduce(
                out=den0[:], in_=esc0, axis=mybir.AxisListType.X, op=ALU.add
            )
            rec0 = strm2.tile([P, 1], F32, tag="rec0")
            nc.vector.reciprocal(rec0[:], den0[:])
            S["rec0"] = rec0
            den1 = wrk.tile([P, K], F32, tag="den1")
            nc.vector.tensor_reduce(
                out=den1[:],
                in_=esc1.rearrange("p (m n) -> p m n", n=K),
                axis=mybir.AxisListType.X,
                op=ALU.add,
            )
            rc1 = wrk.tile([P, K], F32, tag="rc1")
            nc.vector.reciprocal(rc1[:], den1[:])
            e1w = wrk.tile([P, K, K], F32, tag="e1w")
            nc.vector.tensor_tensor(
                out=e1w[:],
                in0=esc1.rearrange("p (m n) -> p m n", n=K),
                in1=rc1[:].broadcast_to([P, K, K]),
                op=ALU.mult,
            )
            e1wb = wrk.tile([P, K * K], BF16, tag="e1wb")
            nc.scalar.activation(e1wb[:], e1w[:].rearrange("p m n -> p (m n)"), AF.Copy)
            for s in range(PACK):
                nc.vector.tensor_tensor(
                    out=w4t[:, :, s], in0=w4t[:, :, s], in1=e1wb[:], op=ALU.mult
                )

            # iter-0 hop-0: x0 -> h0 (per-tile matmul)
            wv0 = wrk.tile([P, K, D], F32, tag="wev0")
            nc.vector.tensor_tensor(
                out=wv0[:],
                in0=ev1s[:],
                in1=esc0.broadcast_to([P, K, D]),
                op=ALU.mult,
            )
            agg = wrk.tile([P, D], F32, tag="agg0")
            nc.vector.tensor_reduce(
                out=agg[:],
                in_=wv0[:].rearrange("p n d -> p d n"),
                axis=mybir.AxisListType.X,
                op=ALU.add,
            )
            x0 = wrk.tile([P, D], F32, tag="x0")
            nc.vector.scalar_tensor_tensor(
                out=x0[:], in0=agg[:], scalar=rec0[:, 0:1], in1=ev0[:],
                op0=ALU.mult, op1=ALU.add,
            )
            pst = psA.tile([128, P], F32, tag="t")
            nc.tensor.transpose(pst[:D, :], x0[:], ident[:])
            x0Tt = wrk.tile([D, P], F32, tag="x0Tt")
            nc.scalar.activation(x0Tt[:], pst[:D, :], AF.Copy)
            pm0 = psM.tile([D, 512], F32, tag="mm")
            nc.tensor.matmul(pm0[:, :P], lhsT=wt_sb[:], rhs=x0Tt[:], start=True, stop=True)
            h0T = wrk.tile([D, P], F32, tag="h0T")
            nc.scalar.activation(h0T[:], pm0[:, :P], AF.Sigmoid, bias=bias_sb[:, 0:1])
            pbt = psA.tile([128, P], F32, tag="t")
            nc.tensor.transpose(pbt[:, :D], h0T[:], ident[:D, :D])
            h0 = strm2.tile([P, D], F32, tag="h0")
            nc.scalar.activation(h0[:], pbt[:, :D], AF.Copy)
            S["h0"] = h0

        def heavy(i):
            S = st[i]
            rep2, w4t, ev1s = S["rep2"], S["w4t"], S["ev1s"]

            x1 = strm1.tile([P, K, D], F32, tag="x1")
            nc.scalar.activation(x1[:], ev1s[:], AF.Copy)
            for m in range(K):
                # one full m-group: two 1024-descriptor gathers into one tile
                g = gat.tile([P, K, PACK * D], BF16, tag="g")
                for h in range(2):
                    fat_gather(
                        g[:, h * 8 : (h + 1) * 8, :], e4_d[:],
                        rep2[:, (2 * m + h) * 64 : (2 * m + h + 1) * 64],
                        GIDX, PACK * D,
                    )
                wv = wev.tile([P, K * PACK, D], BF16, tag="wev1")
                nc.vector.tensor_tensor(
                    out=wv[:],
                    in0=g[:].rearrange("p s (f d) -> p (s f) d", d=D),
                    in1=w4t[:, m * K : (m + 1) * K, :]
                    .rearrange("p t f -> p (t f)")
                    .broadcast_to([P, K * PACK, D]),
                    op=ALU.mult,
                )
                # contiguous-run tree reduction over the 64 (t,s) slots; a
                # single strided tensor_reduce (stride 128B) runs ~6x slower
                w = K * PACK
                while w > 1:
                    h = w // 2
                    nc.vector.tensor_tensor(
                        out=wv[:, 0:h, :], in0=wv[:, 0:h, :], in1=wv[:, h:w, :],
                        op=ALU.add,
                    )
                    w = h
                nc.vector.tensor_tensor(
                    out=x1[:, m, :], in0=x1[:, m, :], in1=wv[:, 0, :], op=ALU.add
                )

            x1T = strm1.tile([D, K * P], F32, tag="x1T")
            for m in range(K):
                pst = psA.tile([128, P], F32, tag="t")
                nc.tensor.transpose(pst[:D, :], x1[:, m, :], ident[:])
                nc.scalar.activation(x1T[:, m * P : (m + 1) * P], pst[:D, :], AF.Copy)
            h1T = strm1.tile([D, K * P], F32, tag="h1T")
            for j in range(K * P // 512):
                pm = psM.tile([D, 512], F32, tag="mm")
                nc.tensor.matmul(
                    pm[:], lhsT=wt_sb[:], rhs=x1T[:, j * 512 : (j + 1) * 512],
                    start=True, stop=True,
                )
                nc.scalar.activation(
                    h1T[:, j * 512 : (j + 1) * 512], pm[:], AF.Sigmoid,
                    bias=bias_sb[:, 0:1],
                )
            h1 = strm1.tile([P, K, D], F32, tag="h1")
            for m in range(K):
                pbt = psA.tile([128, P], F32, tag="t")
                nc.tensor.transpose(pbt[:, :D], h1T[:, m * P : (m + 1) * P], ident[:D, :D])
                nc.scalar.activation(h1[:, m, :], pbt[:, :D], AF.Copy)

            # iter-1 hop-0 -> xfT columns
            wv = strm1.tile([P, K, D], F32, tag="wevf")
            nc.vector.tensor_tensor(
                out=wv[:],
                in0=h1[:],
                in1=S["esc0"].broadcast_to([P, K, D]),
                op=ALU.mult,
            )
            agg = wrk.tile([P, D], F32, tag="aggf")
            nc.vector.tensor_reduce(
                out=agg[:],
                in_=wv[:].rearrange("p n d -> p d n"),
                axis=mybir.AxisListType.X,
                op=ALU.add,
            )
            xf = wrk.tile([P, D], F32, tag="xf")
            nc.vector.scalar_tensor_tensor(
                out=xf[:], in0=agg[:], scalar=S["rec0"][:, 0:1], in1=S["h0"][:],
                op0=ALU.mult, op1=ALU.add,
            )
            pst = psA.tile([128, P], F32, tag="t")
            nc.tensor.transpose(pst[:D, :], xf[:], ident[:])
            nc.scalar.activation(xfT[:, i * P : (i + 1) * P], pst[:D, :], AF.Copy)

        # software pipeline: frontend(i+1) overlaps heavy(i)
        frontend(0)
        for i in range(nt):
            if i + 1 < nt:
                frontend(i + 1)
            heavy(i)

        # ================= final: tanh linear + user.item =================
        pmf = psM.tile([D, bl], F32, tag="mm")
        nc.tensor.matmul(pmf[:], lhsT=wt_sb[:], rhs=xfT[:], start=True, stop=True)
        fT = wrk.tile([D, bl], F32, tag="fT")
        nc.scalar.activation(fT[:], pmf[:], AF.Tanh, bias=bias_sb[:, 0:1])
        prod = wrk.tile([D, bl], F32, tag="prod")
        nc.vector.tensor_mul(prod[:], fT[:], userT[:])
        pr = psM.tile([1, bl], F32, tag="pr")
        nc.tensor.matmul(pr[:], lhsT=ones64[:], rhs=prod[:], start=True, stop=True)
        out_sb = wrk.tile([1, bl], F32, tag="out_sb")
        nc.scalar.activation(out_sb[:], pr[:], AF.Sigmoid)
        nc.sync.dma_start(out=out_d[:].rearrange("(one b) -> one b", one=1), in_=out_sb[:])

    # Spread Pool-engine DMAs over the 4 SWDGE queues AFTER tile scheduling
    # (walking the final instruction order). CoreSim's sem-queue-lock model
    # rejects this (the framework's sem resets run on queue 0), but on HW
    # the per-descriptor completion sems fire correctly from any queue —
    # verified empirically. Keep queue 0 for sim validation.
    if spread_queues:
        from concourse.tile_sem_assignment import DMAInst

        lane = 0
        for blk in nc.main_func.blocks:
            for inst in blk.instructions:
                if isinstance(inst, DMAInst) and inst.engine == mybir.EngineType.Pool:
                    q = (lane % 8) % NQ
                    lane += 1
                    if isinstance(inst, mybir.InstDMACopy):
                        inst.queue = f"qPoolDynamic{q}" if q else "qPoolDynamic"
                    else:
                        inst.queue_num = q

    nc.finalize()
    return nc


_program_cache = {}


def _get_program(total=TOTAL, bl=BL):
    key = (total, bl)
    if key not in _program_cache:
        _program_cache[key] = build_program(total, bl)
    return _program_cache[key]


def make_in_maps(u, v, adj_ent, adj_rel, entity_embed, rel_embed, W, b, n_cores=N_CORES):
    import ml_dtypes

    bl = u.shape[0] // n_cores
    total = entity_embed.shape[0]
    prows = total // PACK
    ae4 = adj_ent.astype(np.int32).reshape(prows, PACK * K)
    ar4 = adj_rel.astype(np.int32).reshape(prows, PACK * K)
    acmb = np.ascontiguousarray(np.concatenate([ae4, ar4], axis=1))
    entf = np.ascontiguousarray(entity_embed.astype(np.float32))
    e4 = np.ascontiguousarray(
        entf.reshape(prows, PACK * D).astype(ml_dtypes.bfloat16)
    )
    relT = np.ascontiguousarray(rel_embed.astype(np.float32).T)
    wt = np.ascontiguousarray(W.astype(np.float32).T)
    bias = np.ascontiguousarray(b.astype(np.float32))
    u32 = u.astype(np.int32)
    v32 = v.astype(np.int32)
    vq32 = (v32 >> 2).astype(np.int32)
    svf = (v32 & 3).astype(np.float32)
    return [
        {
            "u32": np.ascontiguousarray(u32[c * bl : (c + 1) * bl]),
            "v32": np.ascontiguousarray(v32[c * bl : (c + 1) * bl]),
            "vq32": np.ascontiguousarray(vq32[c * bl : (c + 1) * bl]),
            "svf": np.ascontiguousarray(svf[c * bl : (c + 1) * bl]),
            "acmb": acmb,
            "e4": e4,
            "ent": entf,
            "relT": relT,
            "Wt": wt,
            "bias": bias,
        }
        for c in range(n_cores)
    ]


def kernel(u, v, adj_ent, adj_rel, entity_embed, rel_embed, W, b, **run_kwargs):
    u = np.asarray(u)
    v = np.asarray(v)
    nc = _get_program(np.asarray(entity_embed).shape[0], u.shape[0] // N_CORES)
    in_maps = make_in_maps(
        u, v, np.asarray(adj_ent), np.asarray(adj_rel),
        np.asarray(entity_embed), np.asarray(rel_embed), np.asarray(W), np.asarray(b),
    )
    res = run_bass_kernel_spmd(nc, in_maps, core_ids=list(range(N_CORES)), **run_kwargs)
    out = np.concatenate([res.results[c]["out"] for c in range(N_CORES)])
    if run_kwargs.get("trace"):
        return out, res
    return out


# revision 19
# speedup vs baseline: 1.2760x; 1.2760x over previous
"""KGCN (2-hop, 16-neighbor, relation-attention GNN) forward on 8 Trainium2 NeuronCores.

Data-parallel over batch (512 rows/core); tables replicated per core.

The dominant cost is ~140k random embedding-row fetches per core. SWDGE
descriptor generation on the GPSIMD engine costs ~7ns/descriptor no matter
the instruction, so the kernel minimizes per-instruction overhead by using
InstDMAGatherAnt (dma_gather) with 1024 descriptors per instruction spread
over 4 SWDGE queues, instead of one-index-per-partition indirect DMAs
(~1.15us per 128 descriptors).

dma_gather takes int16 indices, so tables > 32768 rows are host-packed 4
rows per table row (index = id>>2 < 27500; 512B descriptors):
  - e4:   entity embeddings [27500, 4*64] bf16 (the wanted 64-f32 sub-row is
          selected by folding a (s == id&3) one-hot into aggregation weights)
  - acmb: adj_ent||adj_rel combo [27500, 4*16 + 4*16] int32 (one gather
          fetches both neighbor ids and relation ids; sub-row selected on DVE)

dma_gather's index layout is (s p)-wrapped over 16 partitions and replicated
x8 (one copy per Q7 core): gather i reads idx[i%16, i//16] and writes output
partition i%128. Tokens are ordered (slot, batch-row) so output partition ==
batch row; the wrapped index buffers are built with PE-transpose pipelines
(partition-crossing element shuffles are only free on the PE).

Execution is software-pipelined per 128-row b-tile: the front end of b-tile
i+1 (adjacency fetch, index wrapping, score selection) is emitted before the
heavy phase of b-tile i (32 embedding gathers + weighted aggregation +
linear), so the GPSIMD engine streams descriptors continuously.

u/v/user lookups (tiny, f32-exact) stay on the old indirect-DMA path.
All index arithmetic runs in f32 (exact for ints < 2^24): sub-row s = x mod
4, packed row = (x - s)/4.
"""

import sys

sys.path.insert(0, "/opt/trn_rl_repo")

from contextlib import ExitStack

import numpy as np

import concourse.bass as bass
import concourse.mybir as mybir
import concourse.tile as tile
from concourse import bacc
from concourse.bass_utils import run_bass_kernel_spmd
from concourse.masks import make_identity

F32 = mybir.dt.float32
BF16 = mybir.dt.bfloat16
I32 = mybir.dt.int32
I16 = mybir.dt.int16
AF = mybir.ActivationFunctionType
ALU = mybir.AluOpType

N_CORES = 8
BATCH = 4096
BL = BATCH // N_CORES  # 512 batch rows per core
P = 128  # partitions
NT = BL // P  # 4 b-tiles per core
K = 16  # neighbors per node
D = 64  # embedding dim
R = 32  # num relations
TOTAL = 110000  # entity table rows (users + entities)
PACK = 4
PROWS = TOTAL // PACK  # 27500 packed rows
NQ = 4  # SWDGE queues
GIDX = 1024  # descriptors per dma_gather instruction (HW ring limit)


def build_program(total=TOTAL, bl=BL, spread_queues=True):
    nt = bl // P
    prows = total // PACK
    nc = bacc.Bacc("TRN2", target_bir_lowering=False, num_swdge_queues=NQ)

    u_d = nc.dram_tensor("u32", [bl], I32, kind="ExternalInput")
    v_d = nc.dram_tensor("v32", [bl], I32, kind="ExternalInput")
    vq_d = nc.dram_tensor("vq32", [bl], I32, kind="ExternalInput")
    sv_d = nc.dram_tensor("svf", [bl], F32, kind="ExternalInput")
    acmb_d = nc.dram_tensor("acmb", [prows, 8 * K], I32, kind="ExternalInput")
    e4_d = nc.dram_tensor("e4", [prows, PACK * D], BF16, kind="ExternalInput")
    ent_d = nc.dram_tensor("ent", [total, D], F32, kind="ExternalInput")
    relT_d = nc.dram_tensor("relT", [D, R], F32, kind="ExternalInput")
    wt_d = nc.dram_tensor("Wt", [D, D], F32, kind="ExternalInput")
    bias_d = nc.dram_tensor("bias", [D], F32, kind="ExternalInput")
    out_d = nc.dram_tensor("out", [bl], F32, kind="ExternalOutput")

    def old_gather(out_ap, table_ap, idx_ap):
        nc.gpsimd.indirect_dma_start(
            out=out_ap,
            out_offset=None,
            in_=table_ap,
            in_offset=bass.IndirectOffsetOnAxis(ap=idx_ap, axis=0),
        )

    def fat_gather(out_ap, table_ap, idx_ap, n_idx, elem):
        nc.gpsimd.dma_gather(
            out_ap=out_ap,
            in_ap=table_ap,
            idxs_ap=idx_ap,
            num_idxs=n_idx,
            num_idxs_reg=n_idx,
            elem_size=elem,
            queue_num=0,
        )

    with ExitStack() as ctx:
        tc = ctx.enter_context(tile.TileContext(nc))
        const = ctx.enter_context(tc.tile_pool(name="const", bufs=1))
        persist = ctx.enter_context(tc.tile_pool(name="persist", bufs=1))
        idxp = ctx.enter_context(tc.tile_pool(name="idxp", bufs=2))
        wrk = ctx.enter_context(tc.tile_pool(name="wrk", bufs=2))
        strm2 = ctx.enter_context(tc.tile_pool(name="strm2", bufs=2))
        strm1 = ctx.enter_context(tc.tile_pool(name="strm1", bufs=1))
        gat = ctx.enter_context(tc.tile_pool(name="gat", bufs=3))
        wev = ctx.enter_context(tc.tile_pool(name="wev", bufs=2))
        psA = ctx.enter_context(tc.tile_pool(name="psA", bufs=3, space="PSUM"))
        psM = ctx.enter_context(tc.tile_pool(name="psM", bufs=2, space="PSUM"))

        # ---- constants ----
        ident = const.tile([P, P], F32)
        make_identity(nc, ident[:])
        ones64 = const.tile([D, 1], F32)
        nc.vector.memset(ones64[:], 1.0)
        wt_sb = const.tile([D, D], F32)
        nc.sync.dma_start(out=wt_sb[:], in_=wt_d[:])
        relT_sb = const.tile([D, R], F32)
        nc.sync.dma_start(out=relT_sb[:], in_=relT_d[:])
        bias_sb = const.tile([D, 1], F32)
        nc.sync.dma_start(out=bias_sb[:], in_=bias_d.rearrange("(d one) -> d one", one=1))

        # ---- persistent (small) ----
        escb = [persist.tile([P, R], F32, name=f"escb_{i}") for i in range(nt)]
        userT = persist.tile([D, bl], F32, tag="userT")
        xfT = persist.tile([D, bl], F32, tag="xfT")

        def wrap16(src_f32_cols, dst_i16, ncols):
            """dst[q, t*8+g] = src[16g+q, t] (the (s p)-wrapped idx layout)."""
            nchunk = (ncols + 127) // 128
            for c in range(nchunk):
                w = min(128, ncols - c * 128)
                pt = psA.tile([128, P], F32, tag="t")
                nc.tensor.transpose(
                    pt[:w, :], src_f32_cols[:, c * 128 : c * 128 + w], ident[:]
                )
                tcs = wrk.tile([128, P], F32, tag="wrTc")
                nc.scalar.activation(tcs[:w, :], pt[:w, :], AF.Copy)
                for g in range(8):
                    pg = psA.tile([128, P], F32, tag="t")
                    nc.tensor.transpose(
                        pg[:16, :w], tcs[:w, g * 16 : (g + 1) * 16], ident[:w, :w]
                    )
                    st = c * 1024 + g
                    nc.vector.tensor_copy(
                        dst_i16[0:16, st : st + (w - 1) * 8 + 1 : 8],
                        pg[:16, :w],
                    )

        def replicate8(dst_128, src_16, width):
            v = dst_128.rearrange("(r q) c -> r q c", q=16)
            for g in range(8):
                nc.sync.dma_start(out=v[g], in_=src_16[0:16, 0:width])

        # ================= pass A: user embeddings -> userT =================
        for i in range(nt):
            sl = slice(i * P, (i + 1) * P)
            uidx = idxp.tile([P, 1], I32, tag="uidx")
            nc.sync.dma_start(
                out=uidx[:], in_=u_d[sl].rearrange("(p one) -> p one", one=1)
            )
            user_g = wrk.tile([P, D], F32, tag="user_g")
            old_gather(user_g[:], ent_d[:], uidx[:, 0:1])
            pst = psA.tile([128, P], F32, tag="t")
            nc.tensor.transpose(pst[:D, :], user_g[:], ident[:])
            nc.scalar.activation(userT[:, sl], pst[:D, :], AF.Copy)

        # ================= phase 2: relation scores =================
        ps = psM.tile([R, bl], F32, tag="mm")
        nc.tensor.matmul(ps[:], lhsT=relT_sb[:], rhs=userT[:], start=True, stop=True)
        esc_sb = wrk.tile([R, bl], F32, tag="esc_sb")
        nc.scalar.activation(esc_sb[:], ps[:], AF.Exp)
        for i in range(nt):
            pe = psA.tile([128, P], F32, tag="t")
            nc.tensor.transpose(pe[:, :R], esc_sb[:, i * P : (i + 1) * P], ident[:R, :R])
            nc.scalar.activation(escb[i][:], pe[:, :R], AF.Copy)

        # state handed from frontend(i) to heavy(i)
        st = [dict() for _ in range(nt)]

        def frontend(i):
            sl = slice(i * P, (i + 1) * P)
            S = st[i]

            vidx = idxp.tile([P, 1], I32, tag="vidx")
            nc.sync.dma_start(
                out=vidx[:], in_=v_d[sl].rearrange("(p one) -> p one", one=1)
            )
            ev0 = strm2.tile([P, D], F32, tag="ev0")
            old_gather(ev0[:], ent_d[:], vidx[:, 0:1])
            S["ev0"] = ev0

            vq = idxp.tile([P, 1], I32, tag="vq")
            nc.sync.dma_start(
                out=vq[:], in_=vq_d[sl].rearrange("(p one) -> p one", one=1)
            )
            vadj = wrk.tile([P, 8 * K], I32, tag="vadj")
            old_gather(vadj[:], acmb_d[:], vq[:, 0:1])
            svt = idxp.tile([P, 1], F32, tag="svt")
            nc.sync.dma_start(
                out=svt[:], in_=sv_d[sl].rearrange("(p one) -> p one", one=1)
            )

            r01 = strm2.tile([P, K + K * K], F32, tag="r01")
            r0f = r01[:, 0:K]
            r1f = r01[:, K : K + K * K]
            vadjf = wrk.tile([P, 8 * K], F32, tag="vadjf")
            nc.vector.tensor_copy(vadjf[:], vadj[:])
            e1x = wrk.tile([P, K], F32, tag="e1x")
            nc.vector.memset(e1x[:], 0.0)
            nc.vector.memset(r0f, 0.0)
            for s in range(PACK):
                m = wrk.tile([P, 1], F32, tag="svm")
                nc.vector.tensor_scalar(
                    out=m[:], in0=svt[:], scalar1=float(s), scalar2=None,
                    op0=ALU.is_equal,
                )
                nc.vector.scalar_tensor_tensor(
                    out=e1x[:], in0=vadjf[:, s * K : (s + 1) * K], scalar=m[:, 0:1],
                    in1=e1x[:], op0=ALU.mult, op1=ALU.add,
                )
                nc.vector.scalar_tensor_tensor(
                    out=r0f, in0=vadjf[:, 4 * K + s * K : 4 * K + (s + 1) * K],
                    scalar=m[:, 0:1], in1=r0f, op0=ALU.mult, op1=ALU.add,
                )

            e1i = wrk.tile([P, K], I32, tag="e1i")
            nc.vector.tensor_copy(e1i[:], e1x[:])
            s1i = wrk.tile([P, K], I32, tag="s1i")
            nc.vector.tensor_scalar(
                out=s1i[:], in0=e1i[:], scalar1=3, scalar2=None, op0=ALU.bitwise_and
            )
            s1f = wrk.tile([P, K], F32, tag="s1f")
            nc.vector.tensor_copy(s1f[:], s1i[:])
            e1qi = wrk.tile([P, K], I32, tag="e1qi")
            nc.vector.tensor_scalar(
                out=e1qi[:], in0=e1i[:], scalar1=2, scalar2=None,
                op0=ALU.logical_shift_right,
            )
            e1q = wrk.tile([P, K], F32, tag="e1q")
            nc.vector.tensor_copy(e1q[:], e1qi[:])

            wr1 = wrk.tile([16, K * 8], I16, tag="wr1")
            wrap16(e1q[:], wr1[:], K)
            rep1 = strm2.tile([P, K * 8], I16, tag="rep1")
            replicate8(rep1[:], wr1[:], K * 8)

            eadj = strm2.tile([P, K, 8 * K], I32, tag="eadj")
            ev1p = strm2.tile([P, K, PACK * D], BF16, tag="ev1p")
            for h in range(2):
                fat_gather(
                    eadj[:, h * 8 : (h + 1) * 8, :], acmb_d[:],
                    rep1[:, h * 64 : (h + 1) * 64], GIDX, 8 * K,
                )
            for h in range(2):
                fat_gather(
                    ev1p[:, h * 8 : (h + 1) * 8, :], e4_d[:],
                    rep1[:, h * 64 : (h + 1) * 64], GIDX, PACK * D,
                )

            # 4-pack select in int32, straight off the gathered rows
            e2acc = wrk.tile([P, K * K], I32, tag="e2acc")
            r1acc = wrk.tile([P, K * K], I32, tag="r1acc")
            nc.vector.memset(e2acc[:], 0)
            nc.vector.memset(r1acc[:], 0)
            e2v = e2acc[:].rearrange("p (m n) -> p m n", n=K)
            r1v = r1acc[:].rearrange("p (m n) -> p m n", n=K)
            for s in range(PACK):
                mi = wrk.tile([P, K], I32, tag="s1mi")
                nc.vector.tensor_scalar(
                    out=mi[:], in0=s1i[:], scalar1=s, scalar2=None,
                    op0=ALU.is_equal,
                )
                t = wrk.tile([P, K, K], I32, tag="selt")
                nc.vector.tensor_tensor(
                    out=t[:], in0=eadj[:, :, s * K : (s + 1) * K],
                    in1=mi[:].broadcast_to([P, K, K]), op=ALU.mult,
                )
                nc.vector.tensor_tensor(out=e2v, in0=e2v, in1=t[:], op=ALU.add)
                nc.vector.tensor_tensor(
                    out=t[:], in0=eadj[:, :, 4 * K + s * K : 4 * K + (s + 1) * K],
                    in1=mi[:].broadcast_to([P, K, K]), op=ALU.mult,
                )
                nc.vector.tensor_tensor(out=r1v, in0=r1v, in1=t[:], op=ALU.add)
            nc.vector.tensor_copy(r1f, r1acc[:])

            e2i = e2acc
            s2i = wrk.tile([P, K * K], I32, tag="s2i")
            nc.vector.tensor_scalar(
                out=s2i[:], in0=e2i[:], scalar1=3, scalar2=None, op0=ALU.bitwise_and
            )
            s2 = wrk.tile([P, K * K], F32, tag="s2")
            nc.vector.tensor_copy(s2[:], s2i[:])
            e2qi = wrk.tile([P, K * K], I32, tag="e2qi")
            nc.vector.tensor_scalar(
                out=e2qi[:], in0=e2i[:], scalar1=2, scalar2=None,
                op0=ALU.logical_shift_right,
            )
            e2q = wrk.tile([P, K * K], F32, tag="e2q")
            nc.vector.tensor_copy(e2q[:], e2qi[:])
            wr2 = wrk.tile([16, 2048], I16, tag="wr2")
            wrap16(e2q[:], wr2[:], K * K)
            rep2 = strm2.tile([P, 2048], I16, tag="rep2")
            replicate8(rep2[:], wr2[:], 2048)
            S["rep2"] = rep2

            # aggregation weights: one-hot(s2) masks (esc factor applied below)
            w4t = strm2.tile([P, K * K, PACK], BF16, tag="w4t")
            for s in range(PACK):
                m = wrk.tile([P, K * K], F32, tag="s2m")
                nc.vector.tensor_scalar(
                    out=m[:], in0=s2[:], scalar1=float(s), scalar2=None,
                    op0=ALU.is_equal,
                )
                nc.scalar.activation(w4t[:, :, s], m[:], AF.Copy)
            S["w4t"] = w4t

            # ev1 selected embeddings
            w1 = wrk.tile([P, K, PACK], BF16, tag="w1")
            for s in range(PACK):
                m = wrk.tile([P, K], F32, tag="s1m2")
                nc.vector.tensor_scalar(
                    out=m[:], in0=s1f[:], scalar1=float(s), scalar2=None,
                    op0=ALU.is_equal,
                )
                nc.scalar.activation(w1[:, :, s], m[:], AF.Copy)
            wv1 = strm1.tile([P, K * PACK, D], BF16, tag="wv1")
            nc.vector.tensor_tensor(
                out=wv1[:],
                in0=ev1p[:].rearrange("p m (f d) -> p (m f) d", d=D),
                in1=w1[:].rearrange("p m f -> p (m f)").broadcast_to([P, K * PACK, D]),
                op=ALU.mult,
            )
            ev1s = strm2.tile([P, K, D], F32, tag="ev1s")
            nc.vector.tensor_reduce(
                out=ev1s[:],
                in_=wv1[:].rearrange("p (m f) d -> p m d f", f=PACK),
                axis=mybir.AxisListType.X,
                op=ALU.add,
            )
            S["ev1s"] = ev1s

            # esc selection + denominators (esc0 and esc1 in one 272-wide pass)
            esc01 = strm2.tile([P, K + K * K], F32, tag="esc01")
            esc0 = esc01[:, 0:K]
            esc1 = esc01[:, K : K + K * K]
            nc.vector.memset(esc01[:], 0.0)
            for r in range(R):
                m01 = wrk.tile([P, K + K * K], F32, tag="m01")
                nc.vector.tensor_scalar(
                    out=m01[:], in0=r01[:], scalar1=float(r), scalar2=None,
                    op0=ALU.is_equal,
                )
                nc.vector.scalar_tensor_tensor(
                    out=esc01[:], in0=m01[:], scalar=escb[i][:, r : r + 1],
                    in1=esc01[:], op0=ALU.mult, op1=ALU.add,
                )
            S["esc0"] = esc0
            den0 = wrk.tile([P, 1], F32, tag="den0")
            nc.vector.tensor_reduce(
                out=den0[:], in_=esc0, axis=mybir.AxisListType.X, op=ALU.add
            )
            rec0 = strm2.tile([P, 1], F32, tag="rec0")
            nc.vector.reciprocal(rec0[:], den0[:])
            S["rec0"] = rec0
            den1 = wrk.tile([P, K], F32, tag="den1")
            nc.vector.tensor_reduce(
                out=den1[:],
                in_=esc1.rearrange("p (m n) -> p m n", n=K),
                axis=mybir.AxisListType.X,
                op=ALU.add,
            )
            rc1 = wrk.tile([P, K], F32, tag="rc1")
            nc.vector.reciprocal(rc1[:], den1[:])
            e1w = wrk.tile([P, K, K], F32, tag="e1w")
            nc.vector.tensor_tensor(
                out=e1w[:],
                in0=esc1.rearrange("p (m n) -> p m n", n=K),
                in1=rc1[:].broadcast_to([P, K, K]),
                op=ALU.mult,
            )
            e1wb = wrk.tile([P, K * K], BF16, tag="e1wb")
            nc.scalar.activation(e1wb[:], e1w[:].rearrange("p m n -> p (m n)"), AF.Copy)
            for s in range(PACK):
                nc.vector.tensor_tensor(
                    out=w4t[:, :, s], in0=w4t[:, :, s], in1=e1wb[:], op=ALU.mult
                )

            # iter-0 hop-0: x0 -> h0 (per-tile matmul)
            wv0 = wrk.tile([P, K, D], F32, tag="wev0")
            nc.vector.tensor_tensor(
                out=wv0[:],
                in0=ev1s[:],
                in1=esc0.broadcast_to([P, K, D]),
                op=ALU.mult,
            )
            agg = wrk.tile([P, D], F32, tag="agg0")
            nc.vector.tensor_reduce(
                out=agg[:],
                in_=wv0[:].rearrange("p n d -> p d n"),
                axis=mybir.AxisListType.X,
                op=ALU.add,
            )
            x0 = wrk.tile([P, D], F32, tag="x0")
            nc.vector.scalar_tensor_tensor(
                out=x0[:], in0=agg[:], scalar=rec0[:, 0:1], in1=ev0[:],
                op0=ALU.mult, op1=ALU.add,
            )
            pst = psA.tile([128, P], F32, tag="t")
            nc.tensor.transpose(pst[:D, :], x0[:], ident[:])
            x0Tt = wrk.tile([D, P], F32, tag="x0Tt")
            nc.scalar.activation(x0Tt[:], pst[:D, :], AF.Copy)
            pm0 = psM.tile([D, 512], F32, tag="mm")
            nc.tensor.matmul(pm0[:, :P], lhsT=wt_sb[:], rhs=x0Tt[:], start=True, stop=True)
            h0T = wrk.tile([D, P], F32, tag="h0T")
            nc.scalar.activation(h0T[:], pm0[:, :P], AF.Sigmoid, bias=bias_sb[:, 0:1])
            pbt = psA.tile([128, P], F32, tag="t")
            nc.tensor.transpose(pbt[:, :D], h0T[:], ident[:D, :D])
            h0 = strm2.tile([P, D], F32, tag="h0")
            nc.scalar.activation(h0[:], pbt[:, :D], AF.Copy)
            S["h0"] = h0

        def heavy(i):
            S = st[i]
            rep2, w4t, ev1s = S["rep2"], S["w4t"], S["ev1s"]

            x1 = strm1.tile([P, K, D], F32, tag="x1")
            nc.scalar.activation(x1[:], ev1s[:], AF.Copy)
            for m in range(K):
                # one full m-group: two 1024-descriptor gathers into one tile
                g = gat.tile([P, K, PACK * D], BF16, tag="g")
                for h in range(2):
                    fat_gather(
                        g[:, h * 8 : (h + 1) * 8, :], e4_d[:],
                        rep2[:, (2 * m + h) * 64 : (2 * m + h + 1) * 64],
                        GIDX, PACK * D,
                    )
                wv = wev.tile([P, K * PACK, D], BF16, tag="wev1")
                nc.vector.tensor_tensor(
                    out=wv[:],
                    in0=g[:].rearrange("p s (f d) -> p (s f) d", d=D),
                    in1=w4t[:, m * K : (m + 1) * K, :]
                    .rearrange("p t f -> p (t f)")
                    .broadcast_to([P, K * PACK, D]),
                    op=ALU.mult,
                )
                # contiguous-run tree reduction over the 64 (t,s) slots; a
                # single strided tensor_reduce (stride 128B) runs ~6x slower
                w = K * PACK
                while w > 1:
                    h = w // 2
                    nc.vector.tensor_tensor(
                        out=wv[:, 0:h, :], in0=wv[:, 0:h, :], in1=wv[:, h:w, :],
                        op=ALU.add,
                    )
                    w = h
                nc.vector.tensor_tensor(
                    out=x1[:, m, :], in0=x1[:, m, :], in1=wv[:, 0, :], op=ALU.add
                )

            x1T = strm1.tile([D, K * P], F32, tag="x1T")
            for m in range(K):
                pst = psA.tile([128, P], F32, tag="t")
                nc.tensor.transpose(pst[:D, :], x1[:, m, :], ident[:])
                nc.scalar.activation(x1T[:, m * P : (m + 1) * P], pst[:D, :], AF.Copy)
            h1T = strm1.tile([D, K * P], F32, tag="h1T")
            for j in range(K * P // 512):
                pm = psM.tile([D, 512], F32, tag="mm")
                nc.tensor.matmul(
                    pm[:], lhsT=wt_sb[:], rhs=x1T[:, j * 512 : (j + 1) * 512],
                    start=True, stop=True,
                )
                nc.scalar.activation(
                    h1T[:, j * 512 : (j + 1) * 512], pm[:], AF.Sigmoid,
                    bias=bias_sb[:, 0:1],
                )
            h1 = strm1.tile([P, K, D], F32, tag="h1")
            for m in range(K):
                pbt = psA.tile([128, P], F32, tag="t")
                nc.tensor.transpose(pbt[:, :D], h1T[:, m * P : (m + 1) * P], ident[:D, :D])
                nc.scalar.activation(h1[:, m, :], pbt[:, :D], AF.Copy)

            # iter-1 hop-0 -> xfT columns
            wv = strm1.tile([P, K, D], F32, tag="wevf")
            nc.vector.tensor_tensor(
                out=wv[:],
                in0=h1[:],
                in1=S["esc0"].broadcast_to([P, K, D]),
                op=ALU.mult,
            )
            agg = wrk.tile([P, D], F32, tag="aggf")
            nc.vector.tensor_reduce(
                out=agg[:],
                in_=wv[:].rearrange("p n d -> p d n"),
                axis=mybir.AxisListType.X,
                op=ALU.add,
            )
            xf = wrk.tile([P, D], F32, tag="xf")
            nc.vector.scalar_tensor_tensor(
                out=xf[:], in0=agg[:], scalar=S["rec0"][:, 0:1], in1=S["h0"][:],
                op0=ALU.mult, op1=ALU.add,
            )
            pst = psA.tile([128, P], F32, tag="t")
            nc.tensor.transpose(pst[:D, :], xf[:], ident[:])
            nc.scalar.activation(xfT[:, i * P : (i + 1) * P], pst[:D, :], AF.Copy)

        # software pipeline: frontend(i+1) overlaps heavy(i)
        frontend(0)
        for i in range(nt):
            if i + 1 < nt:
                frontend(i + 1)
            heavy(i)

        # ================= final: tanh linear + user.item =================
        pmf = psM.tile([D, bl], F32, tag="mm")
        nc.tensor.matmul(pmf[:], lhsT=wt_sb[:], rhs=xfT[:], start=True, stop=True)
        fT = wrk.tile([D, bl], F32, tag="fT")
        nc.scalar.activation(fT[:], pmf[:], AF.Tanh, bias=bias_sb[:, 0:1])
        prod = wrk.tile([D, bl], F32, tag="prod")
        nc.vector.tensor_mul(prod[:], fT[:], userT[:])
        pr = psM.tile([1, bl], F32, tag="pr")
        nc.tensor.matmul(pr[:], lhsT=ones64[:], rhs=prod[:], start=True, stop=True)
        out_sb = wrk.tile([1, bl], F32, tag="out_sb")
        nc.scalar.activation(out_sb[:], pr[:], AF.Sigmoid)
        nc.sync.dma_start(out=out_d[:].rearrange("(one b) -> one b", one=1), in_=out_sb[:])

    # Spread Pool-engine DMAs over the 4 SWDGE queues AFTER tile scheduling
    # (walking the final instruction order). CoreSim's sem-queue-lock model
    # rejects this (the framework's sem resets run on queue 0), but on HW
    # the per-descriptor completion sems fire correctly from any queue —
    # verified empirically. Keep queue 0 for sim validation.
    if spread_queues:
        from concourse.tile_sem_assignment import DMAInst

        lane = 0
        for blk in nc.main_func.blocks:
            for inst in blk.instructions:
                if isinstance(inst, DMAInst) and inst.engine == mybir.EngineType.Pool:
                    q = (lane % 8) % NQ
                    lane += 1
                    if isinstance(inst, mybir.InstDMACopy):
                        inst.queue = f"qPoolDynamic{q}" if q else "qPoolDynamic"
                    else:
                        inst.queue_num = q

    nc.finalize()
    return nc


_program_cache = {}


def _get_program(total=TOTAL, bl=BL):
    key = (total, bl)
    if key not in _program_cache:
        _program_cache[key] = build_program(total, bl)
    return _program_cache[key]


def make_in_maps(u, v, adj_ent, adj_rel, entity_embed, rel_embed, W, b, n_cores=N_CORES):
    import ml_dtypes

    bl = u.shape[0] // n_cores
    total = entity_embed.shape[0]
    prows = total // PACK
    ae4 = adj_ent.astype(np.int32).reshape(prows, PACK * K)
    ar4 = adj_rel.astype(np.int32).reshape(prows, PACK * K)
    acmb = np.ascontiguousarray(np.concatenate([ae4, ar4], axis=1))
    entf = np.ascontiguousarray(entity_embed.astype(np.float32))
    e4 = np.ascontiguousarray(
        entf.reshape(prows, PACK * D).astype(ml_dtypes.bfloat16)
    )
    relT = np.ascontiguousarray(rel_embed.astype(np.float32).T)
    wt = np.ascontiguousarray(W.astype(np.float32).T)
    bias = np.ascontiguousarray(b.astype(np.float32))
    u32 = u.astype(np.int32)
    v32 = v.astype(np.int32)
    vq32 = (v32 >> 2).astype(np.int32)
    svf = (v32 & 3).astype(np.float32)
    return [
        {
            "u32": np.ascontiguousarray(u32[c * bl : (c + 1) * bl]),
            "v32": np.ascontiguousarray(v32[c * bl : (c + 1) * bl]),
            "vq32": np.ascontiguousarray(vq32[c * bl : (c + 1) * bl]),
            "svf": np.ascontiguousarray(svf[c * bl : (c + 1) * bl]),
            "acmb": acmb,
            "e4": e4,
            "ent": entf,
            "relT": relT,
            "Wt": wt,
            "bias": bias,
        }
        for c in range(n_cores)
    ]


def kernel(u, v, adj_ent, adj_rel, entity_embed, rel_embed, W, b, **run_kwargs):
    u = np.asarray(u)
    v = np.asarray(v)
    nc = _get_program(np.asarray(entity_embed).shape[0], u.shape[0] // N_CORES)
    in_maps = make_in_maps(
        u, v, np.asarray(adj_ent), np.asarray(adj_rel),
        np.asarray(entity_embed), np.asarray(rel_embed), np.asarray(W), np.asarray(b),
    )
    res = run_bass_kernel_spmd(nc, in_maps, core_ids=list(range(N_CORES)), **run_kwargs)
    out = np.concatenate([res.results[c]["out"] for c in range(N_CORES)])
    if run_kwargs.get("trace"):
        return out, res
    return out


# revision 20
# speedup vs baseline: 1.3930x; 1.0917x over previous
"""KGCN (2-hop, 16-neighbor, relation-attention GNN) forward on 8 Trainium2 NeuronCores.

Data-parallel over batch (512 rows/core); tables replicated per core.

The dominant cost is ~140k random embedding-row fetches per core. SWDGE
descriptor generation on the GPSIMD engine costs ~7ns/descriptor no matter
the instruction, so the kernel minimizes per-instruction overhead by using
InstDMAGatherAnt (dma_gather) with 1024 descriptors per instruction spread
over 4 SWDGE queues, instead of one-index-per-partition indirect DMAs
(~1.15us per 128 descriptors).

dma_gather takes int16 indices, so tables > 32768 rows are host-packed 4
rows per table row (index = id>>2 < 27500; 512B descriptors):
  - e4:   entity embeddings [27500, 4*64] bf16 (the wanted 64-f32 sub-row is
          selected by folding a (s == id&3) one-hot into aggregation weights)
  - acmb: adj_ent||adj_rel combo [27500, 4*16 + 4*16] int32 (one gather
          fetches both neighbor ids and relation ids; sub-row selected on DVE)

dma_gather's index layout is (s p)-wrapped over 16 partitions and replicated
x8 (one copy per Q7 core): gather i reads idx[i%16, i//16] and writes output
partition i%128. Tokens are ordered (slot, batch-row) so output partition ==
batch row; the wrapped index buffers are built with PE-transpose pipelines
(partition-crossing element shuffles are only free on the PE).

Execution is software-pipelined per 128-row b-tile: the front end of b-tile
i+1 (adjacency fetch, index wrapping, score selection) is emitted before the
heavy phase of b-tile i (32 embedding gathers + weighted aggregation +
linear), so the GPSIMD engine streams descriptors continuously.

u/v/user lookups (tiny, f32-exact) stay on the old indirect-DMA path.
All index arithmetic runs in f32 (exact for ints < 2^24): sub-row s = x mod
4, packed row = (x - s)/4.
"""

import sys

sys.path.insert(0, "/opt/trn_rl_repo")

from contextlib import ExitStack

import numpy as np

import concourse.bass as bass
import concourse.mybir as mybir
import concourse.tile as tile
from concourse import bacc
from concourse.bass_utils import run_bass_kernel_spmd
from concourse.masks import make_identity

F32 = mybir.dt.float32
BF16 = mybir.dt.bfloat16
I32 = mybir.dt.int32
I16 = mybir.dt.int16
AF = mybir.ActivationFunctionType
ALU = mybir.AluOpType

N_CORES = 8
BATCH = 4096
BL = BATCH // N_CORES  # 512 batch rows per core
P = 128  # partitions
NT = BL // P  # 4 b-tiles per core
K = 16  # neighbors per node
D = 64  # embedding dim
R = 32  # num relations
TOTAL = 110000  # entity table rows (users + entities)
PACK = 4
PROWS = TOTAL // PACK  # 27500 packed rows
NQ = 4  # SWDGE queues
GIDX = 1024  # descriptors per dma_gather instruction (HW ring limit)


def build_program(total=TOTAL, bl=BL, spread_queues=True):
    nt = bl // P
    prows = total // PACK
    nc = bacc.Bacc("TRN2", target_bir_lowering=False, num_swdge_queues=NQ)

    u_d = nc.dram_tensor("u32", [bl], I32, kind="ExternalInput")
    v_d = nc.dram_tensor("v32", [bl], I32, kind="ExternalInput")
    vq_d = nc.dram_tensor("vq32", [bl], I32, kind="ExternalInput")
    sv_d = nc.dram_tensor("svf", [bl], F32, kind="ExternalInput")
    acmb_d = nc.dram_tensor("acmb", [prows, 8 * K], I32, kind="ExternalInput")
    e4_d = nc.dram_tensor("e4", [prows, PACK * D], BF16, kind="ExternalInput")
    ent_d = nc.dram_tensor("ent", [total, D], F32, kind="ExternalInput")
    relT_d = nc.dram_tensor("relT", [D, R], F32, kind="ExternalInput")
    wt_d = nc.dram_tensor("Wt", [D, D], F32, kind="ExternalInput")
    bias_d = nc.dram_tensor("bias", [D], F32, kind="ExternalInput")
    out_d = nc.dram_tensor("out", [bl], F32, kind="ExternalOutput")

    def old_gather(out_ap, table_ap, idx_ap):
        nc.gpsimd.indirect_dma_start(
            out=out_ap,
            out_offset=None,
            in_=table_ap,
            in_offset=bass.IndirectOffsetOnAxis(ap=idx_ap, axis=0),
        )

    def fat_gather(out_ap, table_ap, idx_ap, n_idx, elem):
        nc.gpsimd.dma_gather(
            out_ap=out_ap,
            in_ap=table_ap,
            idxs_ap=idx_ap,
            num_idxs=n_idx,
            num_idxs_reg=n_idx,
            elem_size=elem,
            queue_num=0,
        )

    with ExitStack() as ctx:
        tc = ctx.enter_context(tile.TileContext(nc))
        const = ctx.enter_context(tc.tile_pool(name="const", bufs=1))
        persist = ctx.enter_context(tc.tile_pool(name="persist", bufs=1))
        idxp = ctx.enter_context(tc.tile_pool(name="idxp", bufs=2))
        wrk = ctx.enter_context(tc.tile_pool(name="wrk", bufs=2))
        strm2 = ctx.enter_context(tc.tile_pool(name="strm2", bufs=2))
        strm1 = ctx.enter_context(tc.tile_pool(name="strm1", bufs=1))
        gat = ctx.enter_context(tc.tile_pool(name="gat", bufs=4))
        wev = ctx.enter_context(tc.tile_pool(name="wev", bufs=3))
        psA = ctx.enter_context(tc.tile_pool(name="psA", bufs=3, space="PSUM"))
        psM = ctx.enter_context(tc.tile_pool(name="psM", bufs=2, space="PSUM"))

        # ---- constants ----
        ident = const.tile([P, P], F32)
        make_identity(nc, ident[:])
        ones64 = const.tile([D, 1], F32)
        nc.vector.memset(ones64[:], 1.0)
        wt_sb = const.tile([D, D], F32)
        nc.sync.dma_start(out=wt_sb[:], in_=wt_d[:])
        relT_sb = const.tile([D, R], F32)
        nc.sync.dma_start(out=relT_sb[:], in_=relT_d[:])
        bias_sb = const.tile([D, 1], F32)
        nc.sync.dma_start(out=bias_sb[:], in_=bias_d.rearrange("(d one) -> d one", one=1))

        # ---- persistent (small) ----
        escb = [persist.tile([P, R], F32, name=f"escb_{i}") for i in range(nt)]
        userT = persist.tile([D, bl], F32, tag="userT")
        xfT = persist.tile([D, bl], F32, tag="xfT")

        def wrap16(src_f32_cols, dst_i16, ncols):
            """dst[q, t*8+g] = src[16g+q, t] (the (s p)-wrapped idx layout)."""
            nchunk = (ncols + 127) // 128
            for c in range(nchunk):
                w = min(128, ncols - c * 128)
                pt = psA.tile([128, P], F32, tag="t")
                nc.tensor.transpose(
                    pt[:w, :], src_f32_cols[:, c * 128 : c * 128 + w], ident[:]
                )
                tcs = wrk.tile([128, P], F32, tag="wrTc")
                nc.scalar.activation(tcs[:w, :], pt[:w, :], AF.Copy)
                for g in range(8):
                    pg = psA.tile([128, P], F32, tag="t")
                    nc.tensor.transpose(
                        pg[:16, :w], tcs[:w, g * 16 : (g + 1) * 16], ident[:w, :w]
                    )
                    st = c * 1024 + g
                    nc.vector.tensor_copy(
                        dst_i16[0:16, st : st + (w - 1) * 8 + 1 : 8],
                        pg[:16, :w],
                    )

        def replicate8(dst_128, src_16, width):
            v = dst_128.rearrange("(r q) c -> r q c", q=16)
            for g in range(8):
                nc.sync.dma_start(out=v[g], in_=src_16[0:16, 0:width])

        # ================= pass A: user embeddings -> userT =================
        for i in range(nt):
            sl = slice(i * P, (i + 1) * P)
            uidx = idxp.tile([P, 1], I32, tag="uidx")
            nc.sync.dma_start(
                out=uidx[:], in_=u_d[sl].rearrange("(p one) -> p one", one=1)
            )
            user_g = wrk.tile([P, D], F32, tag="user_g")
            old_gather(user_g[:], ent_d[:], uidx[:, 0:1])
            pst = psA.tile([128, P], F32, tag="t")
            nc.tensor.transpose(pst[:D, :], user_g[:], ident[:])
            nc.scalar.activation(userT[:, sl], pst[:D, :], AF.Copy)

        # ================= phase 2: relation scores =================
        ps = psM.tile([R, bl], F32, tag="mm")
        nc.tensor.matmul(ps[:], lhsT=relT_sb[:], rhs=userT[:], start=True, stop=True)
        esc_sb = wrk.tile([R, bl], F32, tag="esc_sb")
        nc.scalar.activation(esc_sb[:], ps[:], AF.Exp)
        for i in range(nt):
            pe = psA.tile([128, P], F32, tag="t")
            nc.tensor.transpose(pe[:, :R], esc_sb[:, i * P : (i + 1) * P], ident[:R, :R])
            nc.scalar.activation(escb[i][:], pe[:, :R], AF.Copy)

        # state handed from frontend(i) to heavy(i)
        st = [dict() for _ in range(nt)]

        def frontend(i):
            sl = slice(i * P, (i + 1) * P)
            S = st[i]

            vidx = idxp.tile([P, 1], I32, tag="vidx")
            nc.sync.dma_start(
                out=vidx[:], in_=v_d[sl].rearrange("(p one) -> p one", one=1)
            )
            ev0 = strm2.tile([P, D], F32, tag="ev0")
            old_gather(ev0[:], ent_d[:], vidx[:, 0:1])
            S["ev0"] = ev0

            vq = idxp.tile([P, 1], I32, tag="vq")
            nc.sync.dma_start(
                out=vq[:], in_=vq_d[sl].rearrange("(p one) -> p one", one=1)
            )
            vadj = wrk.tile([P, 8 * K], I32, tag="vadj")
            old_gather(vadj[:], acmb_d[:], vq[:, 0:1])
            svt = idxp.tile([P, 1], F32, tag="svt")
            nc.sync.dma_start(
                out=svt[:], in_=sv_d[sl].rearrange("(p one) -> p one", one=1)
            )

            r01 = strm2.tile([P, K + K * K], F32, tag="r01")
            r0f = r01[:, 0:K]
            r1f = r01[:, K : K + K * K]
            vadjf = wrk.tile([P, 8 * K], F32, tag="vadjf")
            nc.vector.tensor_copy(vadjf[:], vadj[:])
            e1x = wrk.tile([P, K], F32, tag="e1x")
            nc.vector.memset(e1x[:], 0.0)
            nc.vector.memset(r0f, 0.0)
            for s in range(PACK):
                m = wrk.tile([P, 1], F32, tag="svm")
                nc.vector.tensor_scalar(
                    out=m[:], in0=svt[:], scalar1=float(s), scalar2=None,
                    op0=ALU.is_equal,
                )
                nc.vector.scalar_tensor_tensor(
                    out=e1x[:], in0=vadjf[:, s * K : (s + 1) * K], scalar=m[:, 0:1],
                    in1=e1x[:], op0=ALU.mult, op1=ALU.add,
                )
                nc.vector.scalar_tensor_tensor(
                    out=r0f, in0=vadjf[:, 4 * K + s * K : 4 * K + (s + 1) * K],
                    scalar=m[:, 0:1], in1=r0f, op0=ALU.mult, op1=ALU.add,
                )

            e1i = wrk.tile([P, K], I32, tag="e1i")
            nc.vector.tensor_copy(e1i[:], e1x[:])
            s1i = wrk.tile([P, K], I32, tag="s1i")
            nc.vector.tensor_scalar(
                out=s1i[:], in0=e1i[:], scalar1=3, scalar2=None, op0=ALU.bitwise_and
            )
            s1f = wrk.tile([P, K], F32, tag="s1f")
            nc.vector.tensor_copy(s1f[:], s1i[:])
            e1qi = wrk.tile([P, K], I32, tag="e1qi")
            nc.vector.tensor_scalar(
                out=e1qi[:], in0=e1i[:], scalar1=2, scalar2=None,
                op0=ALU.logical_shift_right,
            )
            e1q = wrk.tile([P, K], F32, tag="e1q")
            nc.vector.tensor_copy(e1q[:], e1qi[:])

            wr1 = wrk.tile([16, K * 8], I16, tag="wr1")
            wrap16(e1q[:], wr1[:], K)
            rep1 = strm2.tile([P, K * 8], I16, tag="rep1")
            replicate8(rep1[:], wr1[:], K * 8)

            eadj = strm2.tile([P, K, 8 * K], I32, tag="eadj")
            ev1p = strm2.tile([P, K, PACK * D], BF16, tag="ev1p")
            for h in range(2):
                fat_gather(
                    eadj[:, h * 8 : (h + 1) * 8, :], acmb_d[:],
                    rep1[:, h * 64 : (h + 1) * 64], GIDX, 8 * K,
                )
            for h in range(2):
                fat_gather(
                    ev1p[:, h * 8 : (h + 1) * 8, :], e4_d[:],
                    rep1[:, h * 64 : (h + 1) * 64], GIDX, PACK * D,
                )

            # 4-pack select in int32, straight off the gathered rows
            e2acc = wrk.tile([P, K * K], I32, tag="e2acc")
            r1acc = wrk.tile([P, K * K], I32, tag="r1acc")
            nc.vector.memset(e2acc[:], 0)
            nc.vector.memset(r1acc[:], 0)
            e2v = e2acc[:].rearrange("p (m n) -> p m n", n=K)
            r1v = r1acc[:].rearrange("p (m n) -> p m n", n=K)
            for s in range(PACK):
                mi = wrk.tile([P, K], I32, tag="s1mi")
                nc.vector.tensor_scalar(
                    out=mi[:], in0=s1i[:], scalar1=s, scalar2=None,
                    op0=ALU.is_equal,
                )
                t = wrk.tile([P, K, K], I32, tag="selt")
                nc.vector.tensor_tensor(
                    out=t[:], in0=eadj[:, :, s * K : (s + 1) * K],
                    in1=mi[:].broadcast_to([P, K, K]), op=ALU.mult,
                )
                nc.vector.tensor_tensor(out=e2v, in0=e2v, in1=t[:], op=ALU.add)
                nc.vector.tensor_tensor(
                    out=t[:], in0=eadj[:, :, 4 * K + s * K : 4 * K + (s + 1) * K],
                    in1=mi[:].broadcast_to([P, K, K]), op=ALU.mult,
                )
                nc.vector.tensor_tensor(out=r1v, in0=r1v, in1=t[:], op=ALU.add)
            nc.vector.tensor_copy(r1f, r1acc[:])

            e2i = e2acc
            s2i = wrk.tile([P, K * K], I32, tag="s2i")
            nc.vector.tensor_scalar(
                out=s2i[:], in0=e2i[:], scalar1=3, scalar2=None, op0=ALU.bitwise_and
            )
            s2 = wrk.tile([P, K * K], F32, tag="s2")
            nc.vector.tensor_copy(s2[:], s2i[:])
            e2qi = wrk.tile([P, K * K], I32, tag="e2qi")
            nc.vector.tensor_scalar(
                out=e2qi[:], in0=e2i[:], scalar1=2, scalar2=None,
                op0=ALU.logical_shift_right,
            )
            e2q = wrk.tile([P, K * K], F32, tag="e2q")
            nc.vector.tensor_copy(e2q[:], e2qi[:])
            wr2 = wrk.tile([16, 2048], I16, tag="wr2")
            wrap16(e2q[:], wr2[:], K * K)
            rep2 = strm2.tile([P, 2048], I16, tag="rep2")
            replicate8(rep2[:], wr2[:], 2048)
            S["rep2"] = rep2

            # aggregation weights: one-hot(s2) masks (esc factor applied below)
            w4t = strm2.tile([P, K * K, PACK], BF16, tag="w4t")
            for s in range(PACK):
                m = wrk.tile([P, K * K], F32, tag="s2m")
                nc.vector.tensor_scalar(
                    out=m[:], in0=s2[:], scalar1=float(s), scalar2=None,
                    op0=ALU.is_equal,
                )
                nc.scalar.activation(w4t[:, :, s], m[:], AF.Copy)
            S["w4t"] = w4t

            # ev1 selected embeddings
            w1 = wrk.tile([P, K, PACK], BF16, tag="w1")
            for s in range(PACK):
                m = wrk.tile([P, K], F32, tag="s1m2")
                nc.vector.tensor_scalar(
                    out=m[:], in0=s1f[:], scalar1=float(s), scalar2=None,
                    op0=ALU.is_equal,
                )
                nc.scalar.activation(w1[:, :, s], m[:], AF.Copy)
            wv1 = strm1.tile([P, K * PACK, D], BF16, tag="wv1")
            nc.vector.tensor_tensor(
                out=wv1[:],
                in0=ev1p[:].rearrange("p m (f d) -> p (m f) d", d=D),
                in1=w1[:].rearrange("p m f -> p (m f)").broadcast_to([P, K * PACK, D]),
                op=ALU.mult,
            )
            ev1s = strm2.tile([P, K, D], F32, tag="ev1s")
            nc.vector.tensor_reduce(
                out=ev1s[:],
                in_=wv1[:].rearrange("p (m f) d -> p m d f", f=PACK),
                axis=mybir.AxisListType.X,
                op=ALU.add,
            )
            S["ev1s"] = ev1s

            # esc selection + denominators (esc0 and esc1 in one 272-wide pass)
            esc01 = strm2.tile([P, K + K * K], F32, tag="esc01")
            esc0 = esc01[:, 0:K]
            esc1 = esc01[:, K : K + K * K]
            nc.vector.memset(esc01[:], 0.0)
            for r in range(R):
                m01 = wrk.tile([P, K + K * K], F32, tag="m01")
                nc.vector.tensor_scalar(
                    out=m01[:], in0=r01[:], scalar1=float(r), scalar2=None,
                    op0=ALU.is_equal,
                )
                nc.vector.scalar_tensor_tensor(
                    out=esc01[:], in0=m01[:], scalar=escb[i][:, r : r + 1],
                    in1=esc01[:], op0=ALU.mult, op1=ALU.add,
                )
            S["esc0"] = esc0
            den0 = wrk.tile([P, 1], F32, tag="den0")
            nc.vector.tensor_reduce(
                out=den0[:], in_=esc0, axis=mybir.AxisListType.X, op=ALU.add
            )
            rec0 = strm2.tile([P, 1], F32, tag="rec0")
            nc.vector.reciprocal(rec0[:], den0[:])
            S["rec0"] = rec0
            den1 = wrk.tile([P, K], F32, tag="den1")
            nc.vector.tensor_reduce(
                out=den1[:],
                in_=esc1.rearrange("p (m n) -> p m n", n=K),
                axis=mybir.AxisListType.X,
                op=ALU.add,
            )
            rc1 = wrk.tile([P, K], F32, tag="rc1")
            nc.vector.reciprocal(rc1[:], den1[:])
            e1w = wrk.tile([P, K, K], F32, tag="e1w")
            nc.vector.tensor_tensor(
                out=e1w[:],
                in0=esc1.rearrange("p (m n) -> p m n", n=K),
                in1=rc1[:].broadcast_to([P, K, K]),
                op=ALU.mult,
            )
            e1wb = wrk.tile([P, K * K], BF16, tag="e1wb")
            nc.scalar.activation(e1wb[:], e1w[:].rearrange("p m n -> p (m n)"), AF.Copy)
            for s in range(PACK):
                nc.vector.tensor_tensor(
                    out=w4t[:, :, s], in0=w4t[:, :, s], in1=e1wb[:], op=ALU.mult
                )

            # iter-0 hop-0: x0 -> h0 (per-tile matmul)
            wv0 = wrk.tile([P, K, D], F32, tag="wev0")
            nc.vector.tensor_tensor(
                out=wv0[:],
                in0=ev1s[:],
                in1=esc0.broadcast_to([P, K, D]),
                op=ALU.mult,
            )
            agg = wrk.tile([P, D], F32, tag="agg0")
            nc.vector.tensor_reduce(
                out=agg[:],
                in_=wv0[:].rearrange("p n d -> p d n"),
                axis=mybir.AxisListType.X,
                op=ALU.add,
            )
            x0 = wrk.tile([P, D], F32, tag="x0")
            nc.vector.scalar_tensor_tensor(
                out=x0[:], in0=agg[:], scalar=rec0[:, 0:1], in1=ev0[:],
                op0=ALU.mult, op1=ALU.add,
            )
            pst = psA.tile([128, P], F32, tag="t")
            nc.tensor.transpose(pst[:D, :], x0[:], ident[:])
            x0Tt = wrk.tile([D, P], F32, tag="x0Tt")
            nc.scalar.activation(x0Tt[:], pst[:D, :], AF.Copy)
            pm0 = psM.tile([D, 512], F32, tag="mm")
            nc.tensor.matmul(pm0[:, :P], lhsT=wt_sb[:], rhs=x0Tt[:], start=True, stop=True)
            h0T = wrk.tile([D, P], F32, tag="h0T")
            nc.scalar.activation(h0T[:], pm0[:, :P], AF.Sigmoid, bias=bias_sb[:, 0:1])
            pbt = psA.tile([128, P], F32, tag="t")
            nc.tensor.transpose(pbt[:, :D], h0T[:], ident[:D, :D])
            h0 = strm2.tile([P, D], F32, tag="h0")
            nc.scalar.activation(h0[:], pbt[:, :D], AF.Copy)
            S["h0"] = h0

        def heavy(i):
            S = st[i]
            rep2, w4t, ev1s = S["rep2"], S["w4t"], S["ev1s"]

            x1 = strm1.tile([P, K, D], F32, tag="x1")
            nc.scalar.activation(x1[:], ev1s[:], AF.Copy)
            for kk in range(2 * K):
                g = gat.tile([P, 8, PACK * D], BF16, tag="g")
                fat_gather(
                    g[:], e4_d[:], rep2[:, kk * 64 : (kk + 1) * 64], GIDX, PACK * D
                )
                wv = wev.tile([P, 8 * PACK, D], BF16, tag="wev1")
                nc.vector.tensor_tensor(
                    out=wv[:],
                    in0=g[:].rearrange("p s (f d) -> p (s f) d", d=D),
                    in1=w4t[:, kk * 8 : (kk + 1) * 8, :]
                    .rearrange("p t f -> p (t f)")
                    .broadcast_to([P, 8 * PACK, D]),
                    op=ALU.mult,
                )
                # contiguous-run tree reduction over the 32 (t,s) slots; a
                # single strided tensor_reduce (stride 128B) runs ~6x slower
                w = 8 * PACK
                while w > 1:
                    h = w // 2
                    nc.vector.tensor_tensor(
                        out=wv[:, 0:h, :], in0=wv[:, 0:h, :], in1=wv[:, h:w, :],
                        op=ALU.add,
                    )
                    w = h
                m = kk // 2
                nc.vector.tensor_tensor(
                    out=x1[:, m, :], in0=x1[:, m, :], in1=wv[:, 0, :], op=ALU.add
                )

            x1T = strm1.tile([D, K * P], F32, tag="x1T")
            for m in range(K):
                pst = psA.tile([128, P], F32, tag="t")
                nc.tensor.transpose(pst[:D, :], x1[:, m, :], ident[:])
                nc.scalar.activation(x1T[:, m * P : (m + 1) * P], pst[:D, :], AF.Copy)
            h1T = strm1.tile([D, K * P], F32, tag="h1T")
            for j in range(K * P // 512):
                pm = psM.tile([D, 512], F32, tag="mm")
                nc.tensor.matmul(
                    pm[:], lhsT=wt_sb[:], rhs=x1T[:, j * 512 : (j + 1) * 512],
                    start=True, stop=True,
                )
                nc.scalar.activation(
                    h1T[:, j * 512 : (j + 1) * 512], pm[:], AF.Sigmoid,
                    bias=bias_sb[:, 0:1],
                )
            h1 = strm1.tile([P, K, D], F32, tag="h1")
            for m in range(K):
                pbt = psA.tile([128, P], F32, tag="t")
                nc.tensor.transpose(pbt[:, :D], h1T[:, m * P : (m + 1) * P], ident[:D, :D])
                nc.scalar.activation(h1[:, m, :], pbt[:, :D], AF.Copy)

            # iter-1 hop-0 -> xfT columns
            wv = strm1.tile([P, K, D], F32, tag="wevf")
            nc.vector.tensor_tensor(
                out=wv[:],
                in0=h1[:],
                in1=S["esc0"].broadcast_to([P, K, D]),
                op=ALU.mult,
            )
            agg = wrk.tile([P, D], F32, tag="aggf")
            nc.vector.tensor_reduce(
                out=agg[:],
                in_=wv[:].rearrange("p n d -> p d n"),
                axis=mybir.AxisListType.X,
                op=ALU.add,
            )
            xf = wrk.tile([P, D], F32, tag="xf")
            nc.vector.scalar_tensor_tensor(
                out=xf[:], in0=agg[:], scalar=S["rec0"][:, 0:1], in1=S["h0"][:],
                op0=ALU.mult, op1=ALU.add,
            )
            pst = psA.tile([128, P], F32, tag="t")
            nc.tensor.transpose(pst[:D, :], xf[:], ident[:])
            nc.scalar.activation(xfT[:, i * P : (i + 1) * P], pst[:D, :], AF.Copy)

        # software pipeline: frontend(i+1) overlaps heavy(i)
        frontend(0)
        for i in range(nt):
            if i + 1 < nt:
                frontend(i + 1)
            heavy(i)

        # ================= final: tanh linear + user.item =================
        pmf = psM.tile([D, bl], F32, tag="mm")
        nc.tensor.matmul(pmf[:], lhsT=wt_sb[:], rhs=xfT[:], start=True, stop=True)
        fT = wrk.tile([D, bl], F32, tag="fT")
        nc.scalar.activation(fT[:], pmf[:], AF.Tanh, bias=bias_sb[:, 0:1])
        prod = wrk.tile([D, bl], F32, tag="prod")
        nc.vector.tensor_mul(prod[:], fT[:], userT[:])
        pr = psM.tile([1, bl], F32, tag="pr")
        nc.tensor.matmul(pr[:], lhsT=ones64[:], rhs=prod[:], start=True, stop=True)
        out_sb = wrk.tile([1, bl], F32, tag="out_sb")
        nc.scalar.activation(out_sb[:], pr[:], AF.Sigmoid)
        nc.sync.dma_start(out=out_d[:].rearrange("(one b) -> one b", one=1), in_=out_sb[:])

    # Spread Pool-engine DMAs over the 4 SWDGE queues AFTER tile scheduling
    # (walking the final instruction order). CoreSim's sem-queue-lock model
    # rejects this (the framework's sem resets run on queue 0), but on HW
    # the per-descriptor completion sems fire correctly from any queue —
    # verified empirically. Keep queue 0 for sim validation.
    if spread_queues:
        from concourse.tile_sem_assignment import DMAInst

        lane = 0
        for blk in nc.main_func.blocks:
            for inst in blk.instructions:
                if isinstance(inst, DMAInst) and inst.engine == mybir.EngineType.Pool:
                    q = (lane % 8) % NQ
                    lane += 1
                    if isinstance(inst, mybir.InstDMACopy):
                        inst.queue = f"qPoolDynamic{q}" if q else "qPoolDynamic"
                    else:
                        inst.queue_num = q

    nc.finalize()
    return nc


_program_cache = {}


def _get_program(total=TOTAL, bl=BL):
    key = (total, bl)
    if key not in _program_cache:
        _program_cache[key] = build_program(total, bl)
    return _program_cache[key]


def make_in_maps(u, v, adj_ent, adj_rel, entity_embed, rel_embed, W, b, n_cores=N_CORES):
    import ml_dtypes

    bl = u.shape[0] // n_cores
    total = entity_embed.shape[0]
    prows = total // PACK
    ae4 = adj_ent.astype(np.int32).reshape(prows, PACK * K)
    ar4 = adj_rel.astype(np.int32).reshape(prows, PACK * K)
    acmb = np.ascontiguousarray(np.concatenate([ae4, ar4], axis=1))
    entf = np.ascontiguousarray(entity_embed.astype(np.float32))
    e4 = np.ascontiguousarray(
        entf.reshape(prows, PACK * D).astype(ml_dtypes.bfloat16)
    )
    relT = np.ascontiguousarray(rel_embed.astype(np.float32).T)
    wt = np.ascontiguousarray(W.astype(np.float32).T)
    bias = np.ascontiguousarray(b.astype(np.float32))
    u32 = u.astype(np.int32)
    v32 = v.astype(np.int32)
    vq32 = (v32 >> 2).astype(np.int32)
    svf = (v32 & 3).astype(np.float32)
    return [
        {
            "u32": np.ascontiguousarray(u32[c * bl : (c + 1) * bl]),
            "v32": np.ascontiguousarray(v32[c * bl : (c + 1) * bl]),
            "vq32": np.ascontiguousarray(vq32[c * bl : (c + 1) * bl]),
            "svf": np.ascontiguousarray(svf[c * bl : (c + 1) * bl]),
            "acmb": acmb,
            "e4": e4,
            "ent": entf,
            "relT": relT,
            "Wt": wt,
            "bias": bias,
        }
        for c in range(n_cores)
    ]


def kernel(u, v, adj_ent, adj_rel, entity_embed, rel_embed, W, b, **run_kwargs):
    u = np.asarray(u)
    v = np.asarray(v)
    nc = _get_program(np.asarray(entity_embed).shape[0], u.shape[0] // N_CORES)
    in_maps = make_in_maps(
        u, v, np.asarray(adj_ent), np.asarray(adj_rel),
        np.asarray(entity_embed), np.asarray(rel_embed), np.asarray(W), np.asarray(b),
    )
    res = run_bass_kernel_spmd(nc, in_maps, core_ids=list(range(N_CORES)), **run_kwargs)
    out = np.concatenate([res.results[c]["out"] for c in range(N_CORES)])
    if run_kwargs.get("trace"):
        return out, res
    return out


# revision 21
# speedup vs baseline: 1.4651x; 1.0518x over previous
"""KGCN (2-hop, 16-neighbor, relation-attention GNN) forward on 8 Trainium2 NeuronCores.

Data-parallel over batch (512 rows/core); tables replicated per core.

The dominant cost is ~140k random embedding-row fetches per core. SWDGE
descriptor generation on the GPSIMD engine costs ~7ns/descriptor no matter
the instruction, so the kernel minimizes per-instruction overhead by using
InstDMAGatherAnt (dma_gather) with 1024 descriptors per instruction spread
over 4 SWDGE queues, instead of one-index-per-partition indirect DMAs
(~1.15us per 128 descriptors).

dma_gather takes int16 indices, so tables > 32768 rows are host-packed 4
rows per table row (index = id>>2 < 27500; 512B descriptors):
  - e4:   entity embeddings [27500, 4*64] bf16 (the wanted 64-f32 sub-row is
          selected by folding a (s == id&3) one-hot into aggregation weights)
  - acmb: adj_ent||adj_rel combo [27500, 4*16 + 4*16] int32 (one gather
          fetches both neighbor ids and relation ids; sub-row selected on DVE)

dma_gather's index layout is (s p)-wrapped over 16 partitions and replicated
x8 (one copy per Q7 core): gather i reads idx[i%16, i//16] and writes output
partition i%128. Tokens are ordered (slot, batch-row) so output partition ==
batch row; the wrapped index buffers are built with PE-transpose pipelines
(partition-crossing element shuffles are only free on the PE).

Execution is software-pipelined per 128-row b-tile: the front end of b-tile
i+1 (adjacency fetch, index wrapping, score selection) is emitted before the
heavy phase of b-tile i (32 embedding gathers + weighted aggregation +
linear), so the GPSIMD engine streams descriptors continuously.

u/v/user lookups (tiny, f32-exact) stay on the old indirect-DMA path.
All index arithmetic runs in f32 (exact for ints < 2^24): sub-row s = x mod
4, packed row = (x - s)/4.
"""

import sys

sys.path.insert(0, "/opt/trn_rl_repo")

from contextlib import ExitStack

import numpy as np

import concourse.bass as bass
import concourse.mybir as mybir
import concourse.tile as tile
from concourse import bacc
from concourse.bass_utils import run_bass_kernel_spmd
from concourse.masks import make_identity

F32 = mybir.dt.float32
BF16 = mybir.dt.bfloat16
I32 = mybir.dt.int32
I16 = mybir.dt.int16
AF = mybir.ActivationFunctionType
ALU = mybir.AluOpType

N_CORES = 8
BATCH = 4096
BL = BATCH // N_CORES  # 512 batch rows per core
P = 128  # partitions
NT = BL // P  # 4 b-tiles per core
K = 16  # neighbors per node
D = 64  # embedding dim
R = 32  # num relations
TOTAL = 110000  # entity table rows (users + entities)
PACK = 4
PROWS = TOTAL // PACK  # 27500 packed rows
NQ = 4  # SWDGE queues
GIDX = 1024  # descriptors per dma_gather instruction (HW ring limit)


def build_program(total=TOTAL, bl=BL, spread_queues=True):
    nt = bl // P
    prows = total // PACK
    nc = bacc.Bacc("TRN2", target_bir_lowering=False, num_swdge_queues=NQ)

    u_d = nc.dram_tensor("u32", [bl], I32, kind="ExternalInput")
    v_d = nc.dram_tensor("v32", [bl], I32, kind="ExternalInput")
    vq_d = nc.dram_tensor("vq32", [bl], I32, kind="ExternalInput")
    sv_d = nc.dram_tensor("svf", [bl], F32, kind="ExternalInput")
    acmb_d = nc.dram_tensor("acmb", [prows, 8 * K], I32, kind="ExternalInput")
    e4_d = nc.dram_tensor("e4", [prows, PACK * D], BF16, kind="ExternalInput")
    ent_d = nc.dram_tensor("ent", [total, D], F32, kind="ExternalInput")
    relT_d = nc.dram_tensor("relT", [D, R], F32, kind="ExternalInput")
    wt_d = nc.dram_tensor("Wt", [D, D], F32, kind="ExternalInput")
    bias_d = nc.dram_tensor("bias", [D], F32, kind="ExternalInput")
    out_d = nc.dram_tensor("out", [bl], F32, kind="ExternalOutput")

    def old_gather(out_ap, table_ap, idx_ap):
        nc.gpsimd.indirect_dma_start(
            out=out_ap,
            out_offset=None,
            in_=table_ap,
            in_offset=bass.IndirectOffsetOnAxis(ap=idx_ap, axis=0),
        )

    def fat_gather(out_ap, table_ap, idx_ap, n_idx, elem):
        nc.gpsimd.dma_gather(
            out_ap=out_ap,
            in_ap=table_ap,
            idxs_ap=idx_ap,
            num_idxs=n_idx,
            num_idxs_reg=n_idx,
            elem_size=elem,
            queue_num=0,
        )

    with ExitStack() as ctx:
        tc = ctx.enter_context(tile.TileContext(nc))
        const = ctx.enter_context(tc.tile_pool(name="const", bufs=1))
        persist = ctx.enter_context(tc.tile_pool(name="persist", bufs=1))
        idxp = ctx.enter_context(tc.tile_pool(name="idxp", bufs=2))
        wrk = ctx.enter_context(tc.tile_pool(name="wrk", bufs=2))
        strm2 = ctx.enter_context(tc.tile_pool(name="strm2", bufs=2))
        strm1 = ctx.enter_context(tc.tile_pool(name="strm1", bufs=1))
        gat = ctx.enter_context(tc.tile_pool(name="gat", bufs=6))
        wev = ctx.enter_context(tc.tile_pool(name="wev", bufs=3))
        psA = ctx.enter_context(tc.tile_pool(name="psA", bufs=3, space="PSUM"))
        psM = ctx.enter_context(tc.tile_pool(name="psM", bufs=2, space="PSUM"))

        # ---- constants ----
        ident = const.tile([P, P], F32)
        make_identity(nc, ident[:])
        ones64 = const.tile([D, 1], F32)
        nc.vector.memset(ones64[:], 1.0)
        wt_sb = const.tile([D, D], F32)
        nc.sync.dma_start(out=wt_sb[:], in_=wt_d[:])
        relT_sb = const.tile([D, R], F32)
        nc.sync.dma_start(out=relT_sb[:], in_=relT_d[:])
        bias_sb = const.tile([D, 1], F32)
        nc.sync.dma_start(out=bias_sb[:], in_=bias_d.rearrange("(d one) -> d one", one=1))

        # ---- persistent (small) ----
        escb = [persist.tile([P, R], F32, name=f"escb_{i}") for i in range(nt)]
        userT = persist.tile([D, bl], F32, tag="userT")
        xfT = persist.tile([D, bl], F32, tag="xfT")

        def wrap16(src_f32_cols, dst_i16, ncols):
            """dst[q, t*8+g] = src[16g+q, t] (the (s p)-wrapped idx layout)."""
            nchunk = (ncols + 127) // 128
            for c in range(nchunk):
                w = min(128, ncols - c * 128)
                pt = psA.tile([128, P], F32, tag="t")
                nc.tensor.transpose(
                    pt[:w, :], src_f32_cols[:, c * 128 : c * 128 + w], ident[:]
                )
                tcs = wrk.tile([128, P], F32, tag="wrTc")
                nc.scalar.activation(tcs[:w, :], pt[:w, :], AF.Copy)
                for g in range(8):
                    pg = psA.tile([128, P], F32, tag="t")
                    nc.tensor.transpose(
                        pg[:16, :w], tcs[:w, g * 16 : (g + 1) * 16], ident[:w, :w]
                    )
                    st = c * 1024 + g
                    nc.vector.tensor_copy(
                        dst_i16[0:16, st : st + (w - 1) * 8 + 1 : 8],
                        pg[:16, :w],
                    )

        def replicate8(dst_128, src_16, width):
            v = dst_128.rearrange("(r q) c -> r q c", q=16)
            for g in range(8):
                nc.sync.dma_start(out=v[g], in_=src_16[0:16, 0:width])

        # ================= pass A: user embeddings -> userT =================
        for i in range(nt):
            sl = slice(i * P, (i + 1) * P)
            uidx = idxp.tile([P, 1], I32, tag="uidx")
            nc.sync.dma_start(
                out=uidx[:], in_=u_d[sl].rearrange("(p one) -> p one", one=1)
            )
            user_g = wrk.tile([P, D], F32, tag="user_g")
            old_gather(user_g[:], ent_d[:], uidx[:, 0:1])
            pst = psA.tile([128, P], F32, tag="t")
            nc.tensor.transpose(pst[:D, :], user_g[:], ident[:])
            nc.scalar.activation(userT[:, sl], pst[:D, :], AF.Copy)

        # ================= phase 2: relation scores =================
        ps = psM.tile([R, bl], F32, tag="mm")
        nc.tensor.matmul(ps[:], lhsT=relT_sb[:], rhs=userT[:], start=True, stop=True)
        esc_sb = wrk.tile([R, bl], F32, tag="esc_sb")
        nc.scalar.activation(esc_sb[:], ps[:], AF.Exp)
        for i in range(nt):
            pe = psA.tile([128, P], F32, tag="t")
            nc.tensor.transpose(pe[:, :R], esc_sb[:, i * P : (i + 1) * P], ident[:R, :R])
            nc.scalar.activation(escb[i][:], pe[:, :R], AF.Copy)

        # state handed from frontend(i) to heavy(i)
        st = [dict() for _ in range(nt)]

        def frontend(i):
            sl = slice(i * P, (i + 1) * P)
            S = st[i]

            vidx = idxp.tile([P, 1], I32, tag="vidx")
            nc.sync.dma_start(
                out=vidx[:], in_=v_d[sl].rearrange("(p one) -> p one", one=1)
            )
            ev0 = strm2.tile([P, D], F32, tag="ev0")
            old_gather(ev0[:], ent_d[:], vidx[:, 0:1])
            S["ev0"] = ev0

            vq = idxp.tile([P, 1], I32, tag="vq")
            nc.sync.dma_start(
                out=vq[:], in_=vq_d[sl].rearrange("(p one) -> p one", one=1)
            )
            vadj = wrk.tile([P, 8 * K], I32, tag="vadj")
            old_gather(vadj[:], acmb_d[:], vq[:, 0:1])
            svt = idxp.tile([P, 1], F32, tag="svt")
            nc.sync.dma_start(
                out=svt[:], in_=sv_d[sl].rearrange("(p one) -> p one", one=1)
            )

            r01 = strm2.tile([P, K + K * K], F32, tag="r01")
            r0f = r01[:, 0:K]
            r1f = r01[:, K : K + K * K]
            vadjf = wrk.tile([P, 8 * K], F32, tag="vadjf")
            nc.vector.tensor_copy(vadjf[:], vadj[:])
            e1x = wrk.tile([P, K], F32, tag="e1x")
            nc.vector.memset(e1x[:], 0.0)
            nc.vector.memset(r0f, 0.0)
            for s in range(PACK):
                m = wrk.tile([P, 1], F32, tag="svm")
                nc.vector.tensor_scalar(
                    out=m[:], in0=svt[:], scalar1=float(s), scalar2=None,
                    op0=ALU.is_equal,
                )
                nc.vector.scalar_tensor_tensor(
                    out=e1x[:], in0=vadjf[:, s * K : (s + 1) * K], scalar=m[:, 0:1],
                    in1=e1x[:], op0=ALU.mult, op1=ALU.add,
                )
                nc.vector.scalar_tensor_tensor(
                    out=r0f, in0=vadjf[:, 4 * K + s * K : 4 * K + (s + 1) * K],
                    scalar=m[:, 0:1], in1=r0f, op0=ALU.mult, op1=ALU.add,
                )

            e1i = wrk.tile([P, K], I32, tag="e1i")
            nc.vector.tensor_copy(e1i[:], e1x[:])
            s1i = wrk.tile([P, K], I32, tag="s1i")
            nc.vector.tensor_scalar(
                out=s1i[:], in0=e1i[:], scalar1=3, scalar2=None, op0=ALU.bitwise_and
            )
            s1f = wrk.tile([P, K], F32, tag="s1f")
            nc.vector.tensor_copy(s1f[:], s1i[:])
            e1qi = wrk.tile([P, K], I32, tag="e1qi")
            nc.vector.tensor_scalar(
                out=e1qi[:], in0=e1i[:], scalar1=2, scalar2=None,
                op0=ALU.logical_shift_right,
            )
            e1q = wrk.tile([P, K], F32, tag="e1q")
            nc.vector.tensor_copy(e1q[:], e1qi[:])

            wr1 = wrk.tile([16, K * 8], I16, tag="wr1")
            wrap16(e1q[:], wr1[:], K)
            rep1 = strm2.tile([P, K * 8], I16, tag="rep1")
            replicate8(rep1[:], wr1[:], K * 8)

            eadj = strm2.tile([P, K, 8 * K], I32, tag="eadj")
            ev1p = strm2.tile([P, K, PACK * D], BF16, tag="ev1p")
            for h in range(2):
                fat_gather(
                    eadj[:, h * 8 : (h + 1) * 8, :], acmb_d[:],
                    rep1[:, h * 64 : (h + 1) * 64], GIDX, 8 * K,
                )
            for h in range(2):
                fat_gather(
                    ev1p[:, h * 8 : (h + 1) * 8, :], e4_d[:],
                    rep1[:, h * 64 : (h + 1) * 64], GIDX, PACK * D,
                )

            # 4-pack select in int32, straight off the gathered rows
            e2acc = wrk.tile([P, K * K], I32, tag="e2acc")
            r1acc = wrk.tile([P, K * K], I32, tag="r1acc")
            nc.vector.memset(e2acc[:], 0)
            nc.vector.memset(r1acc[:], 0)
            e2v = e2acc[:].rearrange("p (m n) -> p m n", n=K)
            r1v = r1acc[:].rearrange("p (m n) -> p m n", n=K)
            for s in range(PACK):
                mi = wrk.tile([P, K], I32, tag="s1mi")
                nc.vector.tensor_scalar(
                    out=mi[:], in0=s1i[:], scalar1=s, scalar2=None,
                    op0=ALU.is_equal,
                )
                t = wrk.tile([P, K, K], I32, tag="selt")
                nc.vector.tensor_tensor(
                    out=t[:], in0=eadj[:, :, s * K : (s + 1) * K],
                    in1=mi[:].broadcast_to([P, K, K]), op=ALU.mult,
                )
                nc.vector.tensor_tensor(out=e2v, in0=e2v, in1=t[:], op=ALU.add)
                nc.vector.tensor_tensor(
                    out=t[:], in0=eadj[:, :, 4 * K + s * K : 4 * K + (s + 1) * K],
                    in1=mi[:].broadcast_to([P, K, K]), op=ALU.mult,
                )
                nc.vector.tensor_tensor(out=r1v, in0=r1v, in1=t[:], op=ALU.add)
            nc.vector.tensor_copy(r1f, r1acc[:])

            e2i = e2acc
            s2i = wrk.tile([P, K * K], I32, tag="s2i")
            nc.vector.tensor_scalar(
                out=s2i[:], in0=e2i[:], scalar1=3, scalar2=None, op0=ALU.bitwise_and
            )
            s2 = wrk.tile([P, K * K], F32, tag="s2")
            nc.vector.tensor_copy(s2[:], s2i[:])
            e2qi = wrk.tile([P, K * K], I32, tag="e2qi")
            nc.vector.tensor_scalar(
                out=e2qi[:], in0=e2i[:], scalar1=2, scalar2=None,
                op0=ALU.logical_shift_right,
            )
            e2q = wrk.tile([P, K * K], F32, tag="e2q")
            nc.vector.tensor_copy(e2q[:], e2qi[:])
            wr2 = wrk.tile([16, 2048], I16, tag="wr2")
            wrap16(e2q[:], wr2[:], K * K)
            rep2 = strm2.tile([P, 2048], I16, tag="rep2")
            replicate8(rep2[:], wr2[:], 2048)
            S["rep2"] = rep2

            # aggregation weights: one-hot(s2) masks (esc factor applied below)
            w4t = strm2.tile([P, K * K, PACK], BF16, tag="w4t")
            for s in range(PACK):
                m = wrk.tile([P, K * K], F32, tag="s2m")
                nc.vector.tensor_scalar(
                    out=m[:], in0=s2[:], scalar1=float(s), scalar2=None,
                    op0=ALU.is_equal,
                )
                nc.scalar.activation(w4t[:, :, s], m[:], AF.Copy)
            S["w4t"] = w4t

            # ev1 selected embeddings
            w1 = wrk.tile([P, K, PACK], BF16, tag="w1")
            for s in range(PACK):
                m = wrk.tile([P, K], F32, tag="s1m2")
                nc.vector.tensor_scalar(
                    out=m[:], in0=s1f[:], scalar1=float(s), scalar2=None,
                    op0=ALU.is_equal,
                )
                nc.scalar.activation(w1[:, :, s], m[:], AF.Copy)
            wv1 = strm1.tile([P, K * PACK, D], BF16, tag="wv1")
            nc.vector.tensor_tensor(
                out=wv1[:],
                in0=ev1p[:].rearrange("p m (f d) -> p (m f) d", d=D),
                in1=w1[:].rearrange("p m f -> p (m f)").broadcast_to([P, K * PACK, D]),
                op=ALU.mult,
            )
            ev1s = strm2.tile([P, K, D], F32, tag="ev1s")
            nc.vector.tensor_reduce(
                out=ev1s[:],
                in_=wv1[:].rearrange("p (m f) d -> p m d f", f=PACK),
                axis=mybir.AxisListType.X,
                op=ALU.add,
            )
            S["ev1s"] = ev1s

            # esc selection + denominators (esc0 and esc1 in one 272-wide pass)
            esc01 = strm2.tile([P, K + K * K], F32, tag="esc01")
            esc0 = esc01[:, 0:K]
            esc1 = esc01[:, K : K + K * K]
            nc.vector.memset(esc01[:], 0.0)
            for r in range(R):
                m01 = wrk.tile([P, K + K * K], F32, tag="m01")
                nc.vector.tensor_scalar(
                    out=m01[:], in0=r01[:], scalar1=float(r), scalar2=None,
                    op0=ALU.is_equal,
                )
                nc.vector.scalar_tensor_tensor(
                    out=esc01[:], in0=m01[:], scalar=escb[i][:, r : r + 1],
                    in1=esc01[:], op0=ALU.mult, op1=ALU.add,
                )
            S["esc0"] = esc0
            den0 = wrk.tile([P, 1], F32, tag="den0")
            nc.vector.tensor_reduce(
                out=den0[:], in_=esc0, axis=mybir.AxisListType.X, op=ALU.add
            )
            rec0 = strm2.tile([P, 1], F32, tag="rec0")
            nc.vector.reciprocal(rec0[:], den0[:])
            S["rec0"] = rec0
            den1 = wrk.tile([P, K], F32, tag="den1")
            nc.vector.tensor_reduce(
                out=den1[:],
                in_=esc1.rearrange("p (m n) -> p m n", n=K),
                axis=mybir.AxisListType.X,
                op=ALU.add,
            )
            rc1 = wrk.tile([P, K], F32, tag="rc1")
            nc.vector.reciprocal(rc1[:], den1[:])
            e1w = wrk.tile([P, K, K], F32, tag="e1w")
            nc.vector.tensor_tensor(
                out=e1w[:],
                in0=esc1.rearrange("p (m n) -> p m n", n=K),
                in1=rc1[:].broadcast_to([P, K, K]),
                op=ALU.mult,
            )
            e1wb = wrk.tile([P, K * K], BF16, tag="e1wb")
            nc.scalar.activation(e1wb[:], e1w[:].rearrange("p m n -> p (m n)"), AF.Copy)
            for s in range(PACK):
                nc.vector.tensor_tensor(
                    out=w4t[:, :, s], in0=w4t[:, :, s], in1=e1wb[:], op=ALU.mult
                )

            # iter-0 hop-0: x0 -> h0 (per-tile matmul)
            wv0 = wrk.tile([P, K, D], F32, tag="wev0")
            nc.vector.tensor_tensor(
                out=wv0[:],
                in0=ev1s[:],
                in1=esc0.broadcast_to([P, K, D]),
                op=ALU.mult,
            )
            agg = wrk.tile([P, D], F32, tag="agg0")
            nc.vector.tensor_reduce(
                out=agg[:],
                in_=wv0[:].rearrange("p n d -> p d n"),
                axis=mybir.AxisListType.X,
                op=ALU.add,
            )
            x0 = wrk.tile([P, D], F32, tag="x0")
            nc.vector.scalar_tensor_tensor(
                out=x0[:], in0=agg[:], scalar=rec0[:, 0:1], in1=ev0[:],
                op0=ALU.mult, op1=ALU.add,
            )
            pst = psA.tile([128, P], F32, tag="t")
            nc.tensor.transpose(pst[:D, :], x0[:], ident[:])
            x0Tt = wrk.tile([D, P], F32, tag="x0Tt")
            nc.scalar.activation(x0Tt[:], pst[:D, :], AF.Copy)
            pm0 = psM.tile([D, 512], F32, tag="mm")
            nc.tensor.matmul(pm0[:, :P], lhsT=wt_sb[:], rhs=x0Tt[:], start=True, stop=True)
            h0T = wrk.tile([D, P], F32, tag="h0T")
            nc.scalar.activation(h0T[:], pm0[:, :P], AF.Sigmoid, bias=bias_sb[:, 0:1])
            pbt = psA.tile([128, P], F32, tag="t")
            nc.tensor.transpose(pbt[:, :D], h0T[:], ident[:D, :D])
            h0 = strm2.tile([P, D], F32, tag="h0")
            nc.scalar.activation(h0[:], pbt[:, :D], AF.Copy)
            S["h0"] = h0

        def heavy(i):
            S = st[i]
            rep2, w4t, ev1s = S["rep2"], S["w4t"], S["ev1s"]

            x1 = strm1.tile([P, K, D], F32, tag="x1")
            nc.scalar.activation(x1[:], ev1s[:], AF.Copy)
            for kk in range(2 * K):
                g = gat.tile([P, 8, PACK * D], BF16, tag="g")
                fat_gather(
                    g[:], e4_d[:], rep2[:, kk * 64 : (kk + 1) * 64], GIDX, PACK * D
                )
                wv = wev.tile([P, 8 * PACK, D], BF16, tag="wev1")
                nc.vector.tensor_tensor(
                    out=wv[:],
                    in0=g[:].rearrange("p s (f d) -> p (s f) d", d=D),
                    in1=w4t[:, kk * 8 : (kk + 1) * 8, :]
                    .rearrange("p t f -> p (t f)")
                    .broadcast_to([P, 8 * PACK, D]),
                    op=ALU.mult,
                )
                # contiguous-run tree reduction over the 32 (t,s) slots; a
                # single strided tensor_reduce (stride 128B) runs ~6x slower
                w = 8 * PACK
                while w > 1:
                    h = w // 2
                    nc.vector.tensor_tensor(
                        out=wv[:, 0:h, :], in0=wv[:, 0:h, :], in1=wv[:, h:w, :],
                        op=ALU.add,
                    )
                    w = h
                m = kk // 2
                nc.vector.tensor_tensor(
                    out=x1[:, m, :], in0=x1[:, m, :], in1=wv[:, 0, :], op=ALU.add
                )

            x1T = strm1.tile([D, K * P], F32, tag="x1T")
            for m in range(K):
                pst = psA.tile([128, P], F32, tag="t")
                nc.tensor.transpose(pst[:D, :], x1[:, m, :], ident[:])
                nc.scalar.activation(x1T[:, m * P : (m + 1) * P], pst[:D, :], AF.Copy)
            h1T = strm1.tile([D, K * P], F32, tag="h1T")
            for j in range(K * P // 512):
                pm = psM.tile([D, 512], F32, tag="mm")
                nc.tensor.matmul(
                    pm[:], lhsT=wt_sb[:], rhs=x1T[:, j * 512 : (j + 1) * 512],
                    start=True, stop=True,
                )
                nc.scalar.activation(
                    h1T[:, j * 512 : (j + 1) * 512], pm[:], AF.Sigmoid,
                    bias=bias_sb[:, 0:1],
                )
            h1 = strm1.tile([P, K, D], F32, tag="h1")
            for m in range(K):
                pbt = psA.tile([128, P], F32, tag="t")
                nc.tensor.transpose(pbt[:, :D], h1T[:, m * P : (m + 1) * P], ident[:D, :D])
                nc.scalar.activation(h1[:, m, :], pbt[:, :D], AF.Copy)

            # iter-1 hop-0 -> xfT columns
            wv = strm1.tile([P, K, D], F32, tag="wevf")
            nc.vector.tensor_tensor(
                out=wv[:],
                in0=h1[:],
                in1=S["esc0"].broadcast_to([P, K, D]),
                op=ALU.mult,
            )
            agg = wrk.tile([P, D], F32, tag="aggf")
            nc.vector.tensor_reduce(
                out=agg[:],
                in_=wv[:].rearrange("p n d -> p d n"),
                axis=mybir.AxisListType.X,
                op=ALU.add,
            )
            xf = wrk.tile([P, D], F32, tag="xf")
            nc.vector.scalar_tensor_tensor(
                out=xf[:], in0=agg[:], scalar=S["rec0"][:, 0:1], in1=S["h0"][:],
                op0=ALU.mult, op1=ALU.add,
            )
            pst = psA.tile([128, P], F32, tag="t")
            nc.tensor.transpose(pst[:D, :], xf[:], ident[:])
            nc.scalar.activation(xfT[:, i * P : (i + 1) * P], pst[:D, :], AF.Copy)

        # software pipeline: frontend(i+1) overlaps heavy(i)
        frontend(0)
        for i in range(nt):
            if i + 1 < nt:
                frontend(i + 1)
            heavy(i)

        # ================= final: tanh linear + user.item =================
        pmf = psM.tile([D, bl], F32, tag="mm")
        nc.tensor.matmul(pmf[:], lhsT=wt_sb[:], rhs=xfT[:], start=True, stop=True)
        fT = wrk.tile([D, bl], F32, tag="fT")
        nc.scalar.activation(fT[:], pmf[:], AF.Tanh, bias=bias_sb[:, 0:1])
        prod = wrk.tile([D, bl], F32, tag="prod")
        nc.vector.tensor_mul(prod[:], fT[:], userT[:])
        pr = psM.tile([1, bl], F32, tag="pr")
        nc.tensor.matmul(pr[:], lhsT=ones64[:], rhs=prod[:], start=True, stop=True)
        out_sb = wrk.tile([1, bl], F32, tag="out_sb")
        nc.scalar.activation(out_sb[:], pr[:], AF.Sigmoid)
        nc.sync.dma_start(out=out_d[:].rearrange("(one b) -> one b", one=1), in_=out_sb[:])

    # Spread Pool-engine DMAs over the 4 SWDGE queues AFTER tile scheduling
    # (walking the final instruction order). CoreSim's sem-queue-lock model
    # rejects this (the framework's sem resets run on queue 0), but on HW
    # the per-descriptor completion sems fire correctly from any queue —
    # verified empirically. Keep queue 0 for sim validation.
    if spread_queues:
        from concourse.tile_sem_assignment import DMAInst

        lane = 0
        for blk in nc.main_func.blocks:
            for inst in blk.instructions:
                if isinstance(inst, DMAInst) and inst.engine == mybir.EngineType.Pool:
                    q = (lane % 8) % NQ
                    lane += 1
                    if isinstance(inst, mybir.InstDMACopy):
                        inst.queue = f"qPoolDynamic{q}" if q else "qPoolDynamic"
                    else:
                        inst.queue_num = q

    nc.finalize()
    return nc


_program_cache = {}


def _get_program(total=TOTAL, bl=BL):
    key = (total, bl)
    if key not in _program_cache:
        _program_cache[key] = build_program(total, bl)
    return _program_cache[key]


def make_in_maps(u, v, adj_ent, adj_rel, entity_embed, rel_embed, W, b, n_cores=N_CORES):
    import ml_dtypes

    bl = u.shape[0] // n_cores
    total = entity_embed.shape[0]
    prows = total // PACK
    ae4 = adj_ent.astype(np.int32).reshape(prows, PACK * K)
    ar4 = adj_rel.astype(np.int32).reshape(prows, PACK * K)
    acmb = np.ascontiguousarray(np.concatenate([ae4, ar4], axis=1))
    entf = np.ascontiguousarray(entity_embed.astype(np.float32))
    e4 = np.ascontiguousarray(
        entf.reshape(prows, PACK * D).astype(ml_dtypes.bfloat16)
    )
    relT = np.ascontiguousarray(rel_embed.astype(np.float32).T)
    wt = np.ascontiguousarray(W.astype(np.float32).T)
    bias = np.ascontiguousarray(b.astype(np.float32))
    u32 = u.astype(np.int32)
    v32 = v.astype(np.int32)
    vq32 = (v32 >> 2).astype(np.int32)
    svf = (v32 & 3).astype(np.float32)
    return [
        {
            "u32": np.ascontiguousarray(u32[c * bl : (c + 1) * bl]),
            "v32": np.ascontiguousarray(v32[c * bl : (c + 1) * bl]),
            "vq32": np.ascontiguousarray(vq32[c * bl : (c + 1) * bl]),
            "svf": np.ascontiguousarray(svf[c * bl : (c + 1) * bl]),
            "acmb": acmb,
            "e4": e4,
            "ent": entf,
            "relT": relT,
            "Wt": wt,
            "bias": bias,
        }
        for c in range(n_cores)
    ]


def kernel(u, v, adj_ent, adj_rel, entity_embed, rel_embed, W, b, **run_kwargs):
    u = np.asarray(u)
    v = np.asarray(v)
    nc = _get_program(np.asarray(entity_embed).shape[0], u.shape[0] // N_CORES)
    in_maps = make_in_maps(
        u, v, np.asarray(adj_ent), np.asarray(adj_rel),
        np.asarray(entity_embed), np.asarray(rel_embed), np.asarray(W), np.asarray(b),
    )
    res = run_bass_kernel_spmd(nc, in_maps, core_ids=list(range(N_CORES)), **run_kwargs)
    out = np.concatenate([res.results[c]["out"] for c in range(N_CORES)])
    if run_kwargs.get("trace"):
        return out, res
    return out
